# revision 10
# baseline (speedup 1.0000x reference)
"""Trainium2 Bass kernel for the DeepSC transformer block (B=2, G=2048, D=512, H=8).

Sharding: 8 cores = (batch b = core//4) x (query-slice qs = core%4, 512 rows).
Each core computes its 512 query rows of BOTH branches (gene + expr) end to
end; K/V projections over the full sequence are replicated inside each batch
group (no collectives). The host rotates the sequence axis per core so that
the core's own query slice is always chunk 0 -> one SPMD program.

v2 changes vs baseline:
  - host folding: fused projection eliminated (Wf premultiplied into Wq_e /
    Wk_e); key-side biases dropped (softmax-invariant); bv folded into bo.
  - scores for a head-pair land in one [128,1024] PSUM tile -> single exp.
  - reciprocal_approx_fast instead of iterative reciprocal; attention/LN
    epilogues restructured so the PE queue never waits on them.
  - weight-stationary inner loops on K projections (LDWEIGHTS reuse).
  - phase emission order P1g P2g P1e P3g P2e P3e with one global PSUM pool
    (s2 x2 = 4 banks, oacc x2 = 2, acc x2 = 2) so independent phases overlap
    and the PE stays dense (HAM stays warm).
"""
import os
import sys
import time

sys.path.insert(0, "/opt/trn_rl_repo")

import numpy as np
import ml_dtypes

import concourse.bass as bass
import concourse.tile as tile
from concourse import bacc, mybir
from concourse.bass_utils import run_bass_kernel_spmd

F32 = mybir.dt.float32
BF16 = mybir.dt.bfloat16
AF = mybir.ActivationFunctionType
ALU = mybir.AluOpType

B, G, D, H = 2, 2048, 512, 8
HD = D // H          # 64
FF = 4 * D           # 2048
N_CORES = 8
QS = G // 4          # 512 query rows per core
DT = D // 128        # 4 partition tiles over D
GT = G // 128        # 16 partition tiles over G
GC = G // 512        # 4 free-dim chunks over G
FT = FF // 128       # 16 partition tiles over FF

_NVEC_NAMES = [
    ("bq_g", DT), ("bq_e", DT),
    ("bo_g", DT), ("bo_e", DT), ("b2_g", DT), ("b2_e", DT),
    ("gamma_g1", DT), ("beta_g1", DT), ("gamma_g2", DT), ("beta_g2", DT),
    ("gamma_e1", DT), ("beta_e1", DT), ("gamma_e2", DT), ("beta_e2", DT),
    ("b1_g", FT), ("b1_e", FT),
]
_PV_COL = {}
_c = 0
for _n, _t in _NVEC_NAMES:
    _PV_COL[_n] = _c
    _c += _t
PV_NCOL = _c

_DEBUG = bool(os.environ.get("KERNEL_DEBUG"))


def _emit(nc, tc, t):
    def pv(name, i):
        c = _PV_COL[name] + i
        return pvec_sb[:, c:c + 1]

    pools = []

    def open_pool(name, space=None):
        kw = {"space": space} if space else {}
        cm = tc.tile_pool(name=name, bufs=1, **kw)
        pool = cm.__enter__()
        pools.append((name, cm))
        return pool

    def close_pool(name):
        assert pools[-1][0] == name, f"non-LIFO close {name} vs {pools[-1][0]}"
        pools.pop()[1].__exit__(None, None, None)

    pers = open_pool("persist")
    psum = open_pool("psum", space="PSUM")
    work = open_pool("work")
    attn = open_pool("attn")

    pvec_sb = pers.tile([128, PV_NCOL], F32, tag="pvec")
    nc.sync.dma_start(pvec_sb[:], t["pvec"][:])
    ones_r = pers.tile([1, 128], BF16, tag="ones_r")
    nc.vector.memset(ones_r[:], 1.0)
    ones_c = pers.tile([128, 1], BF16, tag="ones_c")
    nc.vector.memset(ones_c[:], 1.0)
    eps_sb = pers.tile([1, 1], F32, tag="eps_sb")
    nc.vector.memset(eps_sb[:], 1e-5)

    mt_sb = []
    for kt in range(GT):
        m = pers.tile([128, QS], BF16, tag="mt", bufs=GT, name=f"mt_{kt}")
        mt_sb.append(m)

    # ---------- P1 weights + embeddings (scoped; freed after P1_e) ----------
    p1pool = open_pool("p1")
    embT = {}
    w_p1 = {}

    def dma_w(pool, dname, ntile, width, tagpfx, bufs=None):
        tiles = []
        for i in range(ntile):
            w = pool.tile([128, width], BF16, tag=tagpfx,
                          bufs=bufs if bufs else ntile, name=f"{tagpfx}_{i}")
            nc.sync.dma_start(w[:], t[dname][i * 128:(i + 1) * 128, :])
            tiles.append(w)
        return tiles

    # DMA in strict need-order: gene P1 first.
    w_p1["wv_g"] = dma_w(p1pool, "wv_g", DT, D, "w_wv")
    w_p1["wk_g"] = dma_w(p1pool, "wk_g", DT, D, "w_wk")
    w_p1["wq_g"] = dma_w(p1pool, "wq_g", DT, D, "w_wq")
    embT["g"] = []
    for dt in range(DT):
        e = p1pool.tile([128, G], BF16, tag="embT", bufs=2 * DT,
                        name=f"embT_g_{dt}")
        nc.sync.dma_start(e[:], t["geneT"][dt * 128:(dt + 1) * 128, :])
        embT["g"].append(e)

    KT = {}
    QT = {}
    V = {}

    def open_attn(br):
        KT[br] = [attn.tile([128, G], BF16, tag="kt", bufs=DT,
                            name=f"KT_{br}_{i}") for i in range(DT)]
        QT[br] = [attn.tile([128, QS], BF16, tag="qt", bufs=DT,
                            name=f"QT_{br}_{i}") for i in range(DT)]
        V[br] = [attn.tile([128, H, HD + 1], BF16, tag="v", bufs=GT,
                           name=f"V_{br}_{i}") for i in range(GT)]

    def emit_p1(br):
        """V/K/Q projections for one branch."""
        wv = w_p1[f"wv_{br}"]
        if br == "g":
            ksrc = [(w_p1["wk_g"][di], embT["g"][di]) for di in range(DT)]
            qsrc = [(w_p1["wq_g"][di], embT["g"][di]) for di in range(DT)]
            bq = "bq_g"
        else:
            ksrc = [(w_p1["wke"][di],
                     embT["g"][di] if di < DT else embT["e"][di - DT])
                    for di in range(2 * DT)]
            qsrc = [(w_p1["wqe"][di],
                     embT["g"][di] if di < DT else embT["e"][di - DT])
                    for di in range(2 * DT)]
            bq = "bq_e"

        # V projection: psv[g_tile, dout] accumulated over di
        for gt in range(GT):
            psv = psum.tile([128, D], F32, tag="acc", bufs=2,
                            name=f"psv_{br}_{gt}")
            for di in range(DT):
                nc.tensor.matmul(
                    psv[:], embT[br][di][:, gt * 128:(gt + 1) * 128],
                    wv[di][:], start=(di == 0), stop=(di == DT - 1))
            nc.vector.tensor_copy(V[br][gt][:, :, 0:HD], psv[:, :])
            nc.vector.memset(V[br][gt][:, :, HD:HD + 1], 1.0)

        # K projection: weight-stationary over gc pairs
        nk = len(ksrc)
        for dt in range(DT):
            for gcp in range(2):
                ps = [psum.tile([128, 512], F32, tag="acc", bufs=2,
                                name=f"psk_{br}_{dt}_{gcp}_{x}")
                      for x in range(2)]
                for di in range(nk):
                    wti, xti = ksrc[di]
                    for g2 in range(2):
                        gc = gcp * 2 + g2
                        nc.tensor.matmul(
                            ps[g2][:], wti[:, dt * 128:(dt + 1) * 128],
                            xti[:, gc * 512:(gc + 1) * 512],
                            start=(di == 0), stop=(di == nk - 1))
                for g2 in range(2):
                    gc = gcp * 2 + g2
                    nc.scalar.copy(
                        KT[br][dt][:, gc * 512:(gc + 1) * 512], ps[g2][:])

        # Q projection (own query slice only) with bias
        for dt in range(DT):
            psq = psum.tile([128, QS], F32, tag="acc", bufs=2,
                            name=f"psq_{br}_{dt}")
            for di in range(len(qsrc)):
                wti, xti = qsrc[di]
                nc.tensor.matmul(
                    psq[:], wti[:, dt * 128:(dt + 1) * 128], xti[:, 0:QS],
                    start=(di == 0), stop=(di == len(qsrc) - 1))
            nc.scalar.activation(QT[br][dt][:], psq[:], AF.Identity,
                                 bias=pv(bq, dt))

    Osc = {}

    def emit_p2(br):
        """Masked attention for one branch -> Osc[br] (normalized, bf16)."""
        Osc[br] = [work.tile([128, QS], BF16, tag="osc", bufs=2 * DT,
                             name=f"Osc_{br}_{i}") for i in range(DT)]
        for hp in range(DT):
            oacc = [psum.tile([HD + 1, QS], F32, tag="oacc", bufs=2,
                              name=f"oacc_{br}_{hp}_{x}") for x in range(2)]
            for kt in range(GT):
                ss = []
                for hh in range(2):
                    lo = hh * 64
                    s = psum.tile([128, QS], F32, tag="s2", bufs=4,
                                  name=f"s_{br}_{hp}_{kt}_{hh}")
                    nc.tensor.matmul(
                        s[:],
                        KT[br][hp][lo:lo + 64, kt * 128:(kt + 1) * 128],
                        QT[br][hp][lo:lo + 64, :],
                        start=True, stop=True, tile_position=(lo, 0))
                    ss.append(s)
                for hh in range(2):
                    e = work.tile([128, QS], BF16, tag="e2", bufs=4,
                                  name=f"e_{br}_{hp}_{kt}_{hh}")
                    nc.scalar.activation(e[:], ss[hh][:], AF.Exp)
                    em = work.tile([128, QS], BF16, tag="em2", bufs=4,
                                   name=f"em_{br}_{hp}_{kt}_{hh}")
                    nc.vector.tensor_tensor(em[:], e[:], mt_sb[kt][:], ALU.mult)
                    nc.tensor.matmul(
                        oacc[hh][:], V[br][kt][:, 2 * hp + hh, :],
                        em[:],
                        start=(kt == 0), stop=(kt == GT - 1))
            # epilogue: normalize by 1/sum (approx reciprocal, off PE path)
            rb = psum.tile([128, QS], F32, tag="acc", bufs=2,
                           name=f"rb_{br}_{hp}")
            for hh in range(2):
                nrm = work.tile([1, QS], F32, tag="nrm", bufs=2,
                                name=f"nrm_{br}_{hp}_{hh}")
                nc.vector.tensor_copy(nrm[:], oacc[hh][HD:HD + 1, :])
                rcp = work.tile([1, QS], F32, tag="rcp", bufs=2,
                                name=f"rcp_{br}_{hp}_{hh}")
                nc.vector.reciprocal_approx_fast(out=rcp[:], in_=nrm[:])
                rcb = work.tile([1, QS], BF16, tag="rcb", bufs=2,
                                name=f"rcb_{br}_{hp}_{hh}")
                nc.vector.tensor_copy(rcb[:], rcp[:])
                nc.tensor.matmul(rb[hh * 64:hh * 64 + 64, :],
                                 ones_r[:, 0:64], rcb[:],
                                 start=True, stop=True)
            rbs = work.tile([128, QS], BF16, tag="rbs", bufs=2,
                            name=f"rbs_{br}_{hp}")
            nc.vector.tensor_copy(rbs[:], rb[:])
            for hh in range(2):
                nc.vector.tensor_tensor(
                    Osc[br][hp][hh * 64:hh * 64 + 64, :],
                    oacc[hh][0:HD, :], rbs[hh * 64:hh * 64 + 64, :], ALU.mult)

    # ---------- P3 (per branch): O proj, LN1, FFN, LN2, out ----------
    def emit_ln(x_tiles, out_writer, tagpfx):
        st_s = psum.tile([1, QS], F32, tag="acc", bufs=2, name=f"sts_{tagpfx}")
        st_q = psum.tile([1, QS], F32, tag="acc", bufs=2, name=f"stq_{tagpfx}")
        for dt in range(DT):
            xb = work.tile([128, QS], BF16, tag="xbf", bufs=2,
                           name=f"xbf_{tagpfx}_{dt}")
            nc.vector.tensor_copy(xb[:], x_tiles[dt][:])
            xq = work.tile([128, QS], BF16, tag="xsq", bufs=2,
                           name=f"xsq_{tagpfx}_{dt}")
            nc.vector.tensor_tensor(xq[:], xb[:], xb[:], ALU.mult)
            nc.tensor.matmul(st_s[:], ones_c[:], xb[:],
                             start=(dt == 0), stop=(dt == DT - 1))
            nc.tensor.matmul(st_q[:], ones_c[:], xq[:],
                             start=(dt == 0), stop=(dt == DT - 1))
        mean = work.tile([1, QS], F32, tag="lnvec", bufs=4,
                         name=f"mean_{tagpfx}")
        nc.scalar.mul(mean[:], st_s[:], 1.0 / D)
        ex2 = work.tile([1, QS], F32, tag="lnvec", bufs=4, name=f"ex2_{tagpfx}")
        nc.scalar.mul(ex2[:], st_q[:], 1.0 / D)
        msq = work.tile([1, QS], F32, tag="lnvec", bufs=4, name=f"msq_{tagpfx}")
        nc.vector.tensor_tensor(msq[:], mean[:], mean[:], ALU.mult)
        var = work.tile([1, QS], F32, tag="lnvec", bufs=4, name=f"var_{tagpfx}")
        nc.vector.tensor_tensor(var[:], ex2[:], msq[:], ALU.subtract)
        lnv = work.tile([1, QS], F32, tag="lnvec", bufs=4, name=f"lnv_{tagpfx}")
        nc.scalar.activation(lnv[:], var[:], AF.Ln, bias=eps_sb[:])
        inv = work.tile([1, QS], F32, tag="lnvec", bufs=4, name=f"inv_{tagpfx}")
        nc.scalar.activation(inv[:], lnv[:], AF.Exp, scale=-0.5)
        mi0 = work.tile([1, QS], BF16, tag="mi", bufs=2, name=f"mi0_{tagpfx}")
        nc.vector.tensor_copy(mi0[:], mean[:])
        mi1 = work.tile([1, QS], BF16, tag="mi", bufs=2, name=f"mi1_{tagpfx}")
        nc.vector.tensor_copy(mi1[:], inv[:])
        mb_ps = psum.tile([128, QS], F32, tag="acc", bufs=2,
                          name=f"mbp_{tagpfx}")
        nc.tensor.matmul(mb_ps[:], ones_r[:], mi0[:], start=True, stop=True)
        mb = work.tile([128, QS], F32, tag="bcs", bufs=2, name=f"mb_{tagpfx}")
        nc.vector.tensor_copy(mb[:], mb_ps[:])
        ib_ps = psum.tile([128, QS], F32, tag="acc", bufs=2,
                          name=f"ibp_{tagpfx}")
        nc.tensor.matmul(ib_ps[:], ones_r[:], mi1[:], start=True, stop=True)
        ib = work.tile([128, QS], F32, tag="bcs", bufs=2, name=f"ib_{tagpfx}")
        nc.vector.tensor_copy(ib[:], ib_ps[:])
        for dt in range(DT):
            t1 = work.tile([128, QS], F32, tag="lt", bufs=2,
                           name=f"lt1_{tagpfx}_{dt}")
            nc.vector.tensor_tensor(t1[:], x_tiles[dt][:], mb[:], ALU.subtract)
            t2 = work.tile([128, QS], F32, tag="lt", bufs=2,
                           name=f"lt2_{tagpfx}_{dt}")
            nc.vector.tensor_tensor(t2[:], t1[:], ib[:], ALU.mult)
            out_writer(dt, t2)

    def emit_p3(br):
        pool = open_pool(f"p3_{br}")
        wo = dma_w(pool, f"wo_{br}", DT, D, "w_wo")
        w1 = dma_w(pool, f"w1_{br}", DT, FF, "w_w1")
        w2 = dma_w(pool, f"w2_{br}", FT, D, "w_w2")
        x1 = []
        for dt in range(DT):
            eq = work.tile([128, QS], BF16, tag="embq", bufs=2,
                           name=f"embq_{br}_{dt}")
            nc.sync.dma_start(eq[:], t[f"embq_{br}"][dt * 128:(dt + 1) * 128, :])
            psy = psum.tile([128, QS], F32, tag="acc", bufs=2,
                            name=f"psy_{br}_{dt}")
            for di in range(DT):
                nc.tensor.matmul(
                    psy[:], wo[di][:, dt * 128:(dt + 1) * 128],
                    Osc[br][di][:], start=(di == 0), stop=(di == DT - 1))
            x = work.tile([128, QS], F32, tag="x", bufs=4,
                          name=f"x1_{br}_{dt}")
            nc.vector.scalar_tensor_tensor(
                x[:], psy[:], pv(f"bo_{br}", dt), eq[:],
                ALU.add, ALU.add)
            x1.append(x)

        h_f, h_bf = [], []
        for dt in range(DT):
            h_f.append(work.tile([128, QS], F32, tag="h_f", bufs=4,
                                 name=f"h_f_{br}_{dt}"))
            h_bf.append(work.tile([128, QS], BF16, tag="h_bf", bufs=4,
                                  name=f"h_bf_{br}_{dt}"))

        def ln1_writer(dt, t2, br=br, h_f=h_f, h_bf=h_bf):
            nc.vector.tensor_scalar(
                h_f[dt][:], t2[:], pv(f"gamma_{br}1", dt),
                pv(f"beta_{br}1", dt), ALU.mult, ALU.add)
            nc.vector.tensor_copy(h_bf[dt][:], h_f[dt][:])

        emit_ln(x1, ln1_writer, f"{br}1")

        gl = []
        for ft in range(FT):
            psu = psum.tile([128, QS], F32, tag="acc", bufs=2,
                            name=f"psu_{br}_{ft}")
            for dt in range(DT):
                nc.tensor.matmul(
                    psu[:], w1[dt][:, ft * 128:(ft + 1) * 128],
                    h_bf[dt][:], start=(dt == 0), stop=(dt == DT - 1))
            g = work.tile([128, QS], BF16, tag="gl", bufs=FT,
                          name=f"gl_{br}_{ft}")
            nc.scalar.activation(g[:], psu[:], AF.Gelu, bias=pv(f"b1_{br}", ft))
            gl.append(g)

        x2 = []
        for dt in range(DT):
            psz = psum.tile([128, QS], F32, tag="acc", bufs=2,
                            name=f"psz_{br}_{dt}")
            for ft in range(FT):
                nc.tensor.matmul(
                    psz[:], w2[ft][:, dt * 128:(dt + 1) * 128],
                    gl[ft][:], start=(ft == 0), stop=(ft == FT - 1))
            x = work.tile([128, QS], F32, tag="x", bufs=4,
                          name=f"x2_{br}_{dt}")
            nc.vector.scalar_tensor_tensor(
                x[:], psz[:], pv(f"b2_{br}", dt), h_f[dt][:],
                ALU.add, ALU.add)
            x2.append(x)

        bi = 0 if br == "g" else 1

        def ln2_writer(dt, t2, br=br, bi=bi):
            o = work.tile([128, QS], F32, tag="ot", bufs=2,
                          name=f"ot_{br}_{dt}")
            nc.vector.tensor_scalar(
                o[:], t2[:], pv(f"gamma_{br}2", dt),
                pv(f"beta_{br}2", dt), ALU.mult, ALU.add)
            nc.sync.dma_start(t["out"][bi][dt * 128:(dt + 1) * 128, :], o[:])

        emit_ln(x2, ln2_writer, f"{br}2")
        close_pool(f"p3_{br}")

    # =================== emission order ===================
    open_attn("g")
    emit_p1("g")

    # DMA for P1_e + mask while P1_g computes
    for kt in range(GT):
        nc.sync.dma_start(mt_sb[kt][:], t["mT"][kt * 128:(kt + 1) * 128, :])
    w_p1["wv_e"] = dma_w(p1pool, "wv_e", DT, D, "w_wv")
    w_p1["wke"] = dma_w(p1pool, "wke", 2 * DT, D, "w_wke")
    w_p1["wqe"] = dma_w(p1pool, "wqe", 2 * DT, D, "w_wqe")
    embT["e"] = []
    for dt in range(DT):
        e = p1pool.tile([128, G], BF16, tag="embT", bufs=2 * DT,
                        name=f"embT_e_{dt}")
        nc.sync.dma_start(e[:], t["exprT"][dt * 128:(dt + 1) * 128, :])
        embT["e"].append(e)

    emit_p2("g")
    open_attn("e")
    emit_p1("e")

    if _DEBUG:
        nc.sync.dma_start(t["dbg_kt"][:], KT["g"][0][:])
        nc.sync.dma_start(t["dbg_qt"][:], QT["g"][0][:])
        nc.sync.dma_start(t["dbg_v"][:], V["g"][0][:])
        nc.sync.dma_start(t["dbg_osc"][:], Osc["g"][0][:])

    close_pool("p1")
    emit_p3("g")
    emit_p2("e")
    emit_p3("e")
    while pools:
        pools.pop()[1].__exit__(None, None, None)


def build_program():
    nc = bacc.Bacc("TRN2", target_bir_lowering=False, debug=False,
                   num_devices=N_CORES)
    t = {}
    t["geneT"] = nc.dram_tensor("geneT", [D, G], BF16, kind="ExternalInput").ap()
    t["exprT"] = nc.dram_tensor("exprT", [D, G], BF16, kind="ExternalInput").ap()
    t["embq_g"] = nc.dram_tensor("embq_g", [D, QS], BF16, kind="ExternalInput").ap()
    t["embq_e"] = nc.dram_tensor("embq_e", [D, QS], BF16, kind="ExternalInput").ap()
    t["mT"] = nc.dram_tensor("mT", [G, QS], BF16, kind="ExternalInput").ap()
    for n in ["wq_g", "wk_g", "wv_g", "wo_g", "wv_e", "wo_e"]:
        t[n] = nc.dram_tensor(n, [D, D], BF16, kind="ExternalInput").ap()
    t["wqe"] = nc.dram_tensor("wqe", [2 * D, D], BF16, kind="ExternalInput").ap()
    t["wke"] = nc.dram_tensor("wke", [2 * D, D], BF16, kind="ExternalInput").ap()
    t["w1_g"] = nc.dram_tensor("w1_g", [D, FF], BF16, kind="ExternalInput").ap()
    t["w1_e"] = nc.dram_tensor("w1_e", [D, FF], BF16, kind="ExternalInput").ap()
    t["w2_g"] = nc.dram_tensor("w2_g", [FF, D], BF16, kind="ExternalInput").ap()
    t["w2_e"] = nc.dram_tensor("w2_e", [FF, D], BF16, kind="ExternalInput").ap()
    t["pvec"] = nc.dram_tensor("pvec", [128, PV_NCOL], F32,
                               kind="ExternalInput").ap()
    t["sel"] = nc.dram_tensor("sel", [2, 128], BF16, kind="ExternalInput").ap()
    t["out"] = nc.dram_tensor("out", [2, D, QS], F32, kind="ExternalOutput").ap()
    if _DEBUG:
        t["dbg_kt"] = nc.dram_tensor("dbg_kt", [128, G], BF16, kind="ExternalOutput").ap()
        t["dbg_qt"] = nc.dram_tensor("dbg_qt", [128, QS], BF16, kind="ExternalOutput").ap()
        t["dbg_v"] = nc.dram_tensor("dbg_v", [128, H, HD + 1], BF16, kind="ExternalOutput").ap()
        t["dbg_osc"] = nc.dram_tensor("dbg_osc", [128, QS], BF16, kind="ExternalOutput").ap()

    with tile.TileContext(nc) as tc:
        _emit(nc, tc, t)
    nc.compile()
    return nc


_NC = None


def _get_nc():
    global _NC
    if _NC is None:
        _NC = build_program()
    return _NC


def _bf(x):
    return np.ascontiguousarray(np.asarray(x, dtype=np.float32).astype(ml_dtypes.bfloat16))


def _f32(x):
    return np.ascontiguousarray(x, dtype=np.float32)


def make_in_maps(ii):
    f = {k: np.asarray(v, np.float32) for k, v in ii.items()}
    # folded weights (host, fp32 precision)
    wqe = (0.125 * f["Wq_e"]) @ f["Wf"]          # (D, 2D)
    bqe = 0.125 * (f["Wq_e"] @ f["bf"] + f["bq_e"])
    wke = f["Wk_e"] @ f["Wf"]                    # (D, 2D)
    bo_g = f["bo_g"] + f["Wo_g"] @ f["bv_gene"]
    bo_e = f["bo_e"] + f["Wo_e"] @ f["bv_expr"]

    shared = {
        "wq_g": _bf((f["Wq_g"] * 0.125).T),
        "wk_g": _bf(f["Wk_g"].T),
        "wv_g": _bf(f["Wv_gene"].T), "wo_g": _bf(f["Wo_g"].T),
        "wv_e": _bf(f["Wv_expr"].T), "wo_e": _bf(f["Wo_e"].T),
        "wqe": _bf(wqe.T), "wke": _bf(wke.T),
        "w1_g": _bf(f["W1_g"].T), "w1_e": _bf(f["W1_e"].T),
        "w2_g": _bf(f["W2_g"].T), "w2_e": _bf(f["W2_e"].T),
    }

    pvec = np.zeros((128, PV_NCOL), np.float32)

    def put(name, vec):
        c = _PV_COL[name]
        v = np.asarray(vec, np.float32)
        for i in range(v.size // 128):
            pvec[:, c + i] = v[i * 128:(i + 1) * 128]

    put("bq_g", f["bq_g"] * 0.125)
    put("bq_e", bqe)
    put("bo_g", bo_g); put("bo_e", bo_e)
    put("b2_g", f["b2_g"]); put("b2_e", f["b2_e"])
    put("gamma_g1", f["gamma_g1"]); put("beta_g1", f["beta_g1"])
    put("gamma_g2", f["gamma_g2"]); put("beta_g2", f["beta_g2"])
    put("gamma_e1", f["gamma_e1"]); put("beta_e1", f["beta_e1"])
    put("gamma_e2", f["gamma_e2"]); put("beta_e2", f["beta_e2"])
    put("b1_g", f["b1_g"]); put("b1_e", f["b1_e"])
    shared["pvec"] = pvec
    selm = np.zeros((2, 128), np.float32)
    selm[0, 0:64] = 1.0
    selm[1, 64:128] = 1.0
    shared["sel"] = _bf(selm)

    in_maps = []
    for core in range(N_CORES):
        b, qs = core // 4, core % 4
        q0 = qs * QS
        geneT = f["gene_emb"][b].T  # (D, G) fp32
        exprT = f["expr_emb"][b].T
        geneT_r = np.roll(geneT, -q0, axis=1)
        exprT_r = np.roll(exprT, -q0, axis=1)
        mt = np.roll(f["M"][b].T[:, q0:q0 + QS], -q0, axis=0)
        im = dict(shared)
        im["geneT"] = _bf(geneT_r)
        im["exprT"] = _bf(exprT_r)
        im["embq_g"] = _bf(geneT[:, q0:q0 + QS])
        im["embq_e"] = _bf(exprT[:, q0:q0 + QS])
        im["mT"] = _bf(mt)
        in_maps.append(im)
    return in_maps


def kernel(**inputs):
    nc = _get_nc()
    ii = {k: np.asarray(v) for k, v in inputs.items()}
    in_maps = make_in_maps(ii)

    trace = bool(os.environ.get("KERNEL_TRACE"))
    res = run_bass_kernel_spmd(nc, in_maps, list(range(N_CORES)), trace=trace)
    if trace:
        kernel.last_exec_time_ns = res.exec_time_ns
        kernel.last_results = res

    out_gene = np.empty((B, G, D), np.float32)
    out_expr = np.empty((B, G, D), np.float32)
    for core in range(N_CORES):
        b, qs = core // 4, core % 4
        q0 = qs * QS
        o = res.results[core]["out"]  # (2, D, QS)
        out_gene[b, q0:q0 + QS, :] = o[0].T
        out_expr[b, q0:q0 + QS, :] = o[1].T
    return out_gene, out_expr


if __name__ == "__main__":
    t0 = time.time()
    _get_nc()
    print(f"program built in {time.time()-t0:.1f}s")


# revision 11
# speedup vs baseline: 1.0625x; 1.0625x over previous
"""Trainium2 Bass kernel for the DeepSC transformer block (B=2, G=2048, D=512, H=8).

Sharding: 8 cores = (batch b = core//4) x (query-slice qs = core%4, 512 rows).
Each core computes its 512 query rows of BOTH branches (gene + expr) end to
end; K/V projections over the full sequence are replicated inside each batch
group (no collectives). The host rotates the sequence axis per core so that
the core's own query slice is always chunk 0 -> one SPMD program.

v2 changes vs baseline:
  - host folding: fused projection eliminated (Wf premultiplied into Wq_e /
    Wk_e); key-side biases dropped (softmax-invariant); bv folded into bo.
  - scores for a head-pair land in one [128,1024] PSUM tile -> single exp.
  - reciprocal_approx_fast instead of iterative reciprocal; attention/LN
    epilogues restructured so the PE queue never waits on them.
  - weight-stationary inner loops on K projections (LDWEIGHTS reuse).
  - phase emission order P1g P2g P1e P3g P2e P3e with one global PSUM pool
    (s2 x2 = 4 banks, oacc x2 = 2, acc x2 = 2) so independent phases overlap
    and the PE stays dense (HAM stays warm).
"""
import os
import sys
import time

sys.path.insert(0, "/opt/trn_rl_repo")

import numpy as np
import ml_dtypes

import concourse.bass as bass
import concourse.tile as tile
from concourse import bacc, mybir
from concourse.bass_utils import run_bass_kernel_spmd

F32 = mybir.dt.float32
BF16 = mybir.dt.bfloat16
AF = mybir.ActivationFunctionType
ALU = mybir.AluOpType

B, G, D, H = 2, 2048, 512, 8
HD = D // H          # 64
FF = 4 * D           # 2048
N_CORES = 8
QS = G // 4          # 512 query rows per core
DT = D // 128        # 4 partition tiles over D
GT = G // 128        # 16 partition tiles over G
GC = G // 512        # 4 free-dim chunks over G
FT = FF // 128       # 16 partition tiles over FF

_NVEC_NAMES = [
    ("bq_g", DT), ("bq_e", DT),
    ("bo_g", DT), ("bo_e", DT), ("b2_g", DT), ("b2_e", DT),
    ("gamma_g1", DT), ("beta_g1", DT), ("gamma_g2", DT), ("beta_g2", DT),
    ("gamma_e1", DT), ("beta_e1", DT), ("gamma_e2", DT), ("beta_e2", DT),
    ("b1_g", FT), ("b1_e", FT),
]
_PV_COL = {}
_c = 0
for _n, _t in _NVEC_NAMES:
    _PV_COL[_n] = _c
    _c += _t
PV_NCOL = _c

_DEBUG = bool(os.environ.get("KERNEL_DEBUG"))


def _emit(nc, tc, t):
    def pv(name, i):
        c = _PV_COL[name] + i
        return pvec_sb[:, c:c + 1]

    pools = []

    def open_pool(name, space=None):
        kw = {"space": space} if space else {}
        cm = tc.tile_pool(name=name, bufs=1, **kw)
        pool = cm.__enter__()
        pools.append((name, cm))
        return pool

    def close_pool(name):
        assert pools[-1][0] == name, f"non-LIFO close {name} vs {pools[-1][0]}"
        pools.pop()[1].__exit__(None, None, None)

    pers = open_pool("persist")
    psum = open_pool("psum", space="PSUM")
    work = open_pool("work")
    attn = open_pool("attn")

    pvec_sb = pers.tile([128, PV_NCOL], F32, tag="pvec")
    nc.sync.dma_start(pvec_sb[:], t["pvec"][:])
    ones_r = pers.tile([1, 128], BF16, tag="ones_r")
    nc.vector.memset(ones_r[:], 1.0)
    ones_c = pers.tile([128, 1], BF16, tag="ones_c")
    nc.vector.memset(ones_c[:], 1.0)
    eps_sb = pers.tile([1, 1], F32, tag="eps_sb")
    nc.vector.memset(eps_sb[:], 1e-5)

    mt_sb = []
    for kt in range(GT):
        m = pers.tile([128, QS], BF16, tag="mt", bufs=GT, name=f"mt_{kt}")
        mt_sb.append(m)

    # ---------- P1 weights + embeddings (scoped; freed after P1_e) ----------
    p1pool = open_pool("p1")
    embT = {}
    w_p1 = {}

    def dma_w(pool, dname, ntile, width, tagpfx, bufs=None):
        tiles = []
        for i in range(ntile):
            w = pool.tile([128, width], BF16, tag=tagpfx,
                          bufs=bufs if bufs else ntile, name=f"{tagpfx}_{i}")
            nc.sync.dma_start(w[:], t[dname][i * 128:(i + 1) * 128, :])
            tiles.append(w)
        return tiles

    # DMA in strict need-order: gene P1 first.
    w_p1["wv_g"] = dma_w(p1pool, "wv_g", DT, D, "w_wv")
    w_p1["wk_g"] = dma_w(p1pool, "wk_g", DT, D, "w_wk")
    w_p1["wq_g"] = dma_w(p1pool, "wq_g", DT, D, "w_wq")
    embT["g"] = []
    for dt in range(DT):
        e = p1pool.tile([128, G], BF16, tag="embT", bufs=2 * DT,
                        name=f"embT_g_{dt}")
        nc.sync.dma_start(e[:], t["geneT"][dt * 128:(dt + 1) * 128, :])
        embT["g"].append(e)

    KT = {}
    QT = {}
    V = {}

    def open_attn(br):
        KT[br] = [attn.tile([128, G], BF16, tag="kt", bufs=DT,
                            name=f"KT_{br}_{i}") for i in range(DT)]
        QT[br] = [attn.tile([128, QS], BF16, tag="qt", bufs=DT,
                            name=f"QT_{br}_{i}") for i in range(DT)]
        V[br] = [attn.tile([128, H, HD + 1], BF16, tag="v", bufs=GT,
                           name=f"V_{br}_{i}") for i in range(GT)]

    def emit_p1(br):
        """V/K/Q projections for one branch."""
        wv = w_p1[f"wv_{br}"]
        if br == "g":
            ksrc = [(w_p1["wk_g"][di], embT["g"][di]) for di in range(DT)]
            qsrc = [(w_p1["wq_g"][di], embT["g"][di]) for di in range(DT)]
            bq = "bq_g"
        else:
            ksrc = [(w_p1["wke"][di],
                     embT["g"][di] if di < DT else embT["e"][di - DT])
                    for di in range(2 * DT)]
            qsrc = [(w_p1["wqe"][di],
                     embT["g"][di] if di < DT else embT["e"][di - DT])
                    for di in range(2 * DT)]
            bq = "bq_e"

        # V projection: psv[g_tile, dout] accumulated over di
        for gt in range(GT):
            psv = psum.tile([128, D], F32, tag="acc", bufs=2,
                            name=f"psv_{br}_{gt}")
            for di in range(DT):
                nc.tensor.matmul(
                    psv[:], embT[br][di][:, gt * 128:(gt + 1) * 128],
                    wv[di][:], start=(di == 0), stop=(di == DT - 1))
            nc.vector.tensor_copy(V[br][gt][:, :, 0:HD], psv[:, :])
            nc.vector.memset(V[br][gt][:, :, HD:HD + 1], 1.0)

        # K projection: weight-stationary over gc pairs
        nk = len(ksrc)
        for dt in range(DT):
            for gcp in range(2):
                ps = [psum.tile([128, 512], F32, tag="acc", bufs=2,
                                name=f"psk_{br}_{dt}_{gcp}_{x}")
                      for x in range(2)]
                for di in range(nk):
                    wti, xti = ksrc[di]
                    for g2 in range(2):
                        gc = gcp * 2 + g2
                        nc.tensor.matmul(
                            ps[g2][:], wti[:, dt * 128:(dt + 1) * 128],
                            xti[:, gc * 512:(gc + 1) * 512],
                            start=(di == 0), stop=(di == nk - 1))
                for g2 in range(2):
                    gc = gcp * 2 + g2
                    nc.vector.tensor_copy(
                        KT[br][dt][:, gc * 512:(gc + 1) * 512], ps[g2][:])

        # Q projection (own query slice only) with bias
        for dt in range(DT):
            psq = psum.tile([128, QS], F32, tag="acc", bufs=2,
                            name=f"psq_{br}_{dt}")
            for di in range(len(qsrc)):
                wti, xti = qsrc[di]
                nc.tensor.matmul(
                    psq[:], wti[:, dt * 128:(dt + 1) * 128], xti[:, 0:QS],
                    start=(di == 0), stop=(di == len(qsrc) - 1))
            nc.scalar.activation(QT[br][dt][:], psq[:], AF.Identity,
                                 bias=pv(bq, dt))

    Osc = {}

    def emit_p2(br):
        """Masked attention for one branch -> Osc[br] (normalized, bf16)."""
        Osc[br] = [work.tile([128, QS], BF16, tag="osc", bufs=2 * DT,
                             name=f"Osc_{br}_{i}") for i in range(DT)]
        for hp in range(DT):
            oacc = [psum.tile([HD + 1, QS], F32, tag="oacc", bufs=2,
                              name=f"oacc_{br}_{hp}_{x}") for x in range(2)]
            for kt in range(GT):
                s2 = psum.tile([128, 2 * QS], F32, tag="s2", bufs=2,
                               name=f"s2_{br}_{hp}_{kt}")
                for hh in range(2):
                    lo = hh * 64
                    nc.tensor.matmul(
                        s2[:, hh * QS:(hh + 1) * QS],
                        KT[br][hp][lo:lo + 64, kt * 128:(kt + 1) * 128],
                        QT[br][hp][lo:lo + 64, :],
                        start=True, stop=True, tile_position=(lo, 0))
                e2 = work.tile([128, 2 * QS], BF16, tag="e2", bufs=2,
                               name=f"e2_{br}_{hp}_{kt}")
                nc.scalar.activation(e2[:], s2[:], AF.Exp)
                em2 = work.tile([128, 2 * QS], BF16, tag="em2", bufs=2,
                                name=f"em2_{br}_{hp}_{kt}")
                nc.vector.tensor_tensor(
                    em2[:, 0:QS], e2[:, 0:QS], mt_sb[kt][:], ALU.mult)
                nc.gpsimd.tensor_tensor(
                    em2[:, QS:2 * QS], e2[:, QS:2 * QS], mt_sb[kt][:],
                    ALU.mult)
                for hh in range(2):
                    nc.tensor.matmul(
                        oacc[hh][:], V[br][kt][:, 2 * hp + hh, :],
                        em2[:, hh * QS:(hh + 1) * QS],
                        start=(kt == 0), stop=(kt == GT - 1))
            # epilogue: normalize by 1/sum (approx reciprocal, off PE path)
            rb = psum.tile([128, QS], F32, tag="s2", bufs=2,
                           name=f"rb_{br}_{hp}")
            for hh in range(2):
                nrm = work.tile([1, QS], F32, tag="nrm", bufs=2,
                                name=f"nrm_{br}_{hp}_{hh}")
                nc.vector.tensor_copy(nrm[:], oacc[hh][HD:HD + 1, :])
                rcp = work.tile([1, QS], F32, tag="rcp", bufs=2,
                                name=f"rcp_{br}_{hp}_{hh}")
                nc.vector.reciprocal_approx_fast(out=rcp[:], in_=nrm[:])
                rcb = work.tile([1, QS], BF16, tag="rcb", bufs=2,
                                name=f"rcb_{br}_{hp}_{hh}")
                nc.vector.tensor_copy(rcb[:], rcp[:])
                nc.tensor.matmul(rb[hh * 64:hh * 64 + 64, :],
                                 ones_r[:, 0:64], rcb[:],
                                 start=True, stop=True)
            rbs = work.tile([128, QS], BF16, tag="rbs", bufs=2,
                            name=f"rbs_{br}_{hp}")
            nc.vector.tensor_copy(rbs[:], rb[:])
            for hh in range(2):
                nc.vector.tensor_tensor(
                    Osc[br][hp][hh * 64:hh * 64 + 64, :],
                    oacc[hh][0:HD, :], rbs[hh * 64:hh * 64 + 64, :], ALU.mult)

    # ---------- P3 (per branch): O proj, LN1, FFN, LN2, out ----------
    def emit_ln(x_tiles, out_writer, tagpfx):
        st_s = psum.tile([1, QS], F32, tag="acc", bufs=2, name=f"sts_{tagpfx}")
        st_q = psum.tile([1, QS], F32, tag="acc", bufs=2, name=f"stq_{tagpfx}")
        for dt in range(DT):
            xb = work.tile([128, QS], BF16, tag="xbf", bufs=2,
                           name=f"xbf_{tagpfx}_{dt}")
            nc.vector.tensor_copy(xb[:], x_tiles[dt][:])
            xq = work.tile([128, QS], BF16, tag="xsq", bufs=2,
                           name=f"xsq_{tagpfx}_{dt}")
            nc.vector.tensor_tensor(xq[:], xb[:], xb[:], ALU.mult)
            nc.tensor.matmul(st_s[:], ones_c[:], xb[:],
                             start=(dt == 0), stop=(dt == DT - 1))
            nc.tensor.matmul(st_q[:], ones_c[:], xq[:],
                             start=(dt == 0), stop=(dt == DT - 1))
        mean = work.tile([1, QS], F32, tag="lnvec", bufs=4,
                         name=f"mean_{tagpfx}")
        nc.scalar.mul(mean[:], st_s[:], 1.0 / D)
        ex2 = work.tile([1, QS], F32, tag="lnvec", bufs=4, name=f"ex2_{tagpfx}")
        nc.scalar.mul(ex2[:], st_q[:], 1.0 / D)
        msq = work.tile([1, QS], F32, tag="lnvec", bufs=4, name=f"msq_{tagpfx}")
        nc.vector.tensor_tensor(msq[:], mean[:], mean[:], ALU.mult)
        var = work.tile([1, QS], F32, tag="lnvec", bufs=4, name=f"var_{tagpfx}")
        nc.vector.tensor_tensor(var[:], ex2[:], msq[:], ALU.subtract)
        sd = work.tile([1, QS], F32, tag="lnvec", bufs=4, name=f"sd_{tagpfx}")
        nc.scalar.activation(sd[:], var[:], AF.Sqrt, bias=eps_sb[:])
        inv = work.tile([1, QS], F32, tag="lnvec", bufs=4, name=f"inv_{tagpfx}")
        nc.vector.reciprocal_approx_fast(out=inv[:], in_=sd[:])
        mi0 = work.tile([1, QS], BF16, tag="mi", bufs=2, name=f"mi0_{tagpfx}")
        nc.vector.tensor_copy(mi0[:], mean[:])
        mi1 = work.tile([1, QS], BF16, tag="mi", bufs=2, name=f"mi1_{tagpfx}")
        nc.vector.tensor_copy(mi1[:], inv[:])
        mb_ps = psum.tile([128, QS], F32, tag="acc", bufs=2,
                          name=f"mbp_{tagpfx}")
        nc.tensor.matmul(mb_ps[:], ones_r[:], mi0[:], start=True, stop=True)
        mb = work.tile([128, QS], F32, tag="bcs", bufs=2, name=f"mb_{tagpfx}")
        nc.vector.tensor_copy(mb[:], mb_ps[:])
        ib_ps = psum.tile([128, QS], F32, tag="acc", bufs=2,
                          name=f"ibp_{tagpfx}")
        nc.tensor.matmul(ib_ps[:], ones_r[:], mi1[:], start=True, stop=True)
        ib = work.tile([128, QS], F32, tag="bcs", bufs=2, name=f"ib_{tagpfx}")
        nc.vector.tensor_copy(ib[:], ib_ps[:])
        for dt in range(DT):
            t1 = work.tile([128, QS], F32, tag="lt", bufs=2,
                           name=f"lt1_{tagpfx}_{dt}")
            nc.vector.tensor_tensor(t1[:], x_tiles[dt][:], mb[:], ALU.subtract)
            t2 = work.tile([128, QS], F32, tag="lt", bufs=2,
                           name=f"lt2_{tagpfx}_{dt}")
            nc.vector.tensor_tensor(t2[:], t1[:], ib[:], ALU.mult)
            out_writer(dt, t2)

    def emit_p3(br):
        pool = open_pool(f"p3_{br}")
        wo = dma_w(pool, f"wo_{br}", DT, D, "w_wo")
        w1 = dma_w(pool, f"w1_{br}", DT, FF, "w_w1")
        w2 = dma_w(pool, f"w2_{br}", FT, D, "w_w2")
        x1 = []
        for dt in range(DT):
            eq = work.tile([128, QS], BF16, tag="embq", bufs=2,
                           name=f"embq_{br}_{dt}")
            nc.sync.dma_start(eq[:], t[f"embq_{br}"][dt * 128:(dt + 1) * 128, :])
            psy = psum.tile([128, QS], F32, tag="acc", bufs=2,
                            name=f"psy_{br}_{dt}")
            for di in range(DT):
                nc.tensor.matmul(
                    psy[:], wo[di][:, dt * 128:(dt + 1) * 128],
                    Osc[br][di][:], start=(di == 0), stop=(di == DT - 1))
            x = work.tile([128, QS], F32, tag="x", bufs=4,
                          name=f"x1_{br}_{dt}")
            nc.vector.scalar_tensor_tensor(
                x[:], psy[:], pv(f"bo_{br}", dt), eq[:],
                ALU.add, ALU.add)
            x1.append(x)

        h_f, h_bf = [], []
        for dt in range(DT):
            h_f.append(work.tile([128, QS], F32, tag="h_f", bufs=4,
                                 name=f"h_f_{br}_{dt}"))
            h_bf.append(work.tile([128, QS], BF16, tag="h_bf", bufs=4,
                                  name=f"h_bf_{br}_{dt}"))

        def ln1_writer(dt, t2, br=br, h_f=h_f, h_bf=h_bf):
            nc.vector.tensor_scalar(
                h_f[dt][:], t2[:], pv(f"gamma_{br}1", dt),
                pv(f"beta_{br}1", dt), ALU.mult, ALU.add)
            nc.vector.tensor_copy(h_bf[dt][:], h_f[dt][:])

        emit_ln(x1, ln1_writer, f"{br}1")

        gl = []
        for ft in range(FT):
            psu = psum.tile([128, QS], F32, tag="acc", bufs=2,
                            name=f"psu_{br}_{ft}")
            for dt in range(DT):
                nc.tensor.matmul(
                    psu[:], w1[dt][:, ft * 128:(ft + 1) * 128],
                    h_bf[dt][:], start=(dt == 0), stop=(dt == DT - 1))
            g = work.tile([128, QS], BF16, tag="gl", bufs=FT,
                          name=f"gl_{br}_{ft}")
            nc.scalar.activation(g[:], psu[:], AF.Gelu, bias=pv(f"b1_{br}", ft))
            gl.append(g)

        x2 = []
        for dt in range(DT):
            psz = psum.tile([128, QS], F32, tag="acc", bufs=2,
                            name=f"psz_{br}_{dt}")
            for ft in range(FT):
                nc.tensor.matmul(
                    psz[:], w2[ft][:, dt * 128:(dt + 1) * 128],
                    gl[ft][:], start=(ft == 0), stop=(ft == FT - 1))
            x = work.tile([128, QS], F32, tag="x", bufs=4,
                          name=f"x2_{br}_{dt}")
            nc.vector.scalar_tensor_tensor(
                x[:], psz[:], pv(f"b2_{br}", dt), h_f[dt][:],
                ALU.add, ALU.add)
            x2.append(x)

        bi = 0 if br == "g" else 1

        def ln2_writer(dt, t2, br=br, bi=bi):
            o = work.tile([128, QS], F32, tag="ot", bufs=2,
                          name=f"ot_{br}_{dt}")
            nc.vector.tensor_scalar(
                o[:], t2[:], pv(f"gamma_{br}2", dt),
                pv(f"beta_{br}2", dt), ALU.mult, ALU.add)
            nc.sync.dma_start(t["out"][bi][dt * 128:(dt + 1) * 128, :], o[:])

        emit_ln(x2, ln2_writer, f"{br}2")
        close_pool(f"p3_{br}")

    # =================== emission order ===================
    open_attn("g")
    emit_p1("g")

    # DMA for P1_e + mask while P1_g computes
    for kt in range(GT):
        nc.sync.dma_start(mt_sb[kt][:], t["mT"][kt * 128:(kt + 1) * 128, :])
    w_p1["wv_e"] = dma_w(p1pool, "wv_e", DT, D, "w_wv")
    w_p1["wke"] = dma_w(p1pool, "wke", 2 * DT, D, "w_wke")
    w_p1["wqe"] = dma_w(p1pool, "wqe", 2 * DT, D, "w_wqe")
    embT["e"] = []
    for dt in range(DT):
        e = p1pool.tile([128, G], BF16, tag="embT", bufs=2 * DT,
                        name=f"embT_e_{dt}")
        nc.sync.dma_start(e[:], t["exprT"][dt * 128:(dt + 1) * 128, :])
        embT["e"].append(e)

    emit_p2("g")
    open_attn("e")
    emit_p1("e")

    if _DEBUG:
        nc.sync.dma_start(t["dbg_kt"][:], KT["g"][0][:])
        nc.sync.dma_start(t["dbg_qt"][:], QT["g"][0][:])
        nc.sync.dma_start(t["dbg_v"][:], V["g"][0][:])
        nc.sync.dma_start(t["dbg_osc"][:], Osc["g"][0][:])

    close_pool("p1")
    emit_p3("g")
    emit_p2("e")
    emit_p3("e")
    while pools:
        pools.pop()[1].__exit__(None, None, None)


def build_program():
    nc = bacc.Bacc("TRN2", target_bir_lowering=False, debug=False,
                   num_devices=N_CORES)
    t = {}
    t["geneT"] = nc.dram_tensor("geneT", [D, G], BF16, kind="ExternalInput").ap()
    t["exprT"] = nc.dram_tensor("exprT", [D, G], BF16, kind="ExternalInput").ap()
    t["embq_g"] = nc.dram_tensor("embq_g", [D, QS], BF16, kind="ExternalInput").ap()
    t["embq_e"] = nc.dram_tensor("embq_e", [D, QS], BF16, kind="ExternalInput").ap()
    t["mT"] = nc.dram_tensor("mT", [G, QS], BF16, kind="ExternalInput").ap()
    for n in ["wq_g", "wk_g", "wv_g", "wo_g", "wv_e", "wo_e"]:
        t[n] = nc.dram_tensor(n, [D, D], BF16, kind="ExternalInput").ap()
    t["wqe"] = nc.dram_tensor("wqe", [2 * D, D], BF16, kind="ExternalInput").ap()
    t["wke"] = nc.dram_tensor("wke", [2 * D, D], BF16, kind="ExternalInput").ap()
    t["w1_g"] = nc.dram_tensor("w1_g", [D, FF], BF16, kind="ExternalInput").ap()
    t["w1_e"] = nc.dram_tensor("w1_e", [D, FF], BF16, kind="ExternalInput").ap()
    t["w2_g"] = nc.dram_tensor("w2_g", [FF, D], BF16, kind="ExternalInput").ap()
    t["w2_e"] = nc.dram_tensor("w2_e", [FF, D], BF16, kind="ExternalInput").ap()
    t["pvec"] = nc.dram_tensor("pvec", [128, PV_NCOL], F32,
                               kind="ExternalInput").ap()
    t["sel"] = nc.dram_tensor("sel", [2, 128], BF16, kind="ExternalInput").ap()
    t["out"] = nc.dram_tensor("out", [2, D, QS], F32, kind="ExternalOutput").ap()
    if _DEBUG:
        t["dbg_kt"] = nc.dram_tensor("dbg_kt", [128, G], BF16, kind="ExternalOutput").ap()
        t["dbg_qt"] = nc.dram_tensor("dbg_qt", [128, QS], BF16, kind="ExternalOutput").ap()
        t["dbg_v"] = nc.dram_tensor("dbg_v", [128, H, HD + 1], BF16, kind="ExternalOutput").ap()
        t["dbg_osc"] = nc.dram_tensor("dbg_osc", [128, QS], BF16, kind="ExternalOutput").ap()

    with tile.TileContext(nc) as tc:
        _emit(nc, tc, t)
    nc.compile()
    return nc


_NC = None


def _get_nc():
    global _NC
    if _NC is None:
        _NC = build_program()
    return _NC


def _bf(x):
    return np.ascontiguousarray(np.asarray(x, dtype=np.float32).astype(ml_dtypes.bfloat16))


def _f32(x):
    return np.ascontiguousarray(x, dtype=np.float32)


def make_in_maps(ii):
    f = {k: np.asarray(v, np.float32) for k, v in ii.items()}
    # folded weights (host, fp32 precision)
    wqe = (0.125 * f["Wq_e"]) @ f["Wf"]          # (D, 2D)
    bqe = 0.125 * (f["Wq_e"] @ f["bf"] + f["bq_e"])
    wke = f["Wk_e"] @ f["Wf"]                    # (D, 2D)
    bo_g = f["bo_g"] + f["Wo_g"] @ f["bv_gene"]
    bo_e = f["bo_e"] + f["Wo_e"] @ f["bv_expr"]

    shared = {
        "wq_g": _bf((f["Wq_g"] * 0.125).T),
        "wk_g": _bf(f["Wk_g"].T),
        "wv_g": _bf(f["Wv_gene"].T), "wo_g": _bf(f["Wo_g"].T),
        "wv_e": _bf(f["Wv_expr"].T), "wo_e": _bf(f["Wo_e"].T),
        "wqe": _bf(wqe.T), "wke": _bf(wke.T),
        "w1_g": _bf(f["W1_g"].T), "w1_e": _bf(f["W1_e"].T),
        "w2_g": _bf(f["W2_g"].T), "w2_e": _bf(f["W2_e"].T),
    }

    pvec = np.zeros((128, PV_NCOL), np.float32)

    def put(name, vec):
        c = _PV_COL[name]
        v = np.asarray(vec, np.float32)
        for i in range(v.size // 128):
            pvec[:, c + i] = v[i * 128:(i + 1) * 128]

    put("bq_g", f["bq_g"] * 0.125)
    put("bq_e", bqe)
    put("bo_g", bo_g); put("bo_e", bo_e)
    put("b2_g", f["b2_g"]); put("b2_e", f["b2_e"])
    put("gamma_g1", f["gamma_g1"]); put("beta_g1", f["beta_g1"])
    put("gamma_g2", f["gamma_g2"]); put("beta_g2", f["beta_g2"])
    put("gamma_e1", f["gamma_e1"]); put("beta_e1", f["beta_e1"])
    put("gamma_e2", f["gamma_e2"]); put("beta_e2", f["beta_e2"])
    put("b1_g", f["b1_g"]); put("b1_e", f["b1_e"])
    shared["pvec"] = pvec
    selm = np.zeros((2, 128), np.float32)
    selm[0, 0:64] = 1.0
    selm[1, 64:128] = 1.0
    shared["sel"] = _bf(selm)

    in_maps = []
    for core in range(N_CORES):
        b, qs = core // 4, core % 4
        q0 = qs * QS
        geneT = f["gene_emb"][b].T  # (D, G) fp32
        exprT = f["expr_emb"][b].T
        geneT_r = np.roll(geneT, -q0, axis=1)
        exprT_r = np.roll(exprT, -q0, axis=1)
        mt = np.roll(f["M"][b].T[:, q0:q0 + QS], -q0, axis=0)
        im = dict(shared)
        im["geneT"] = _bf(geneT_r)
        im["exprT"] = _bf(exprT_r)
        im["embq_g"] = _bf(geneT[:, q0:q0 + QS])
        im["embq_e"] = _bf(exprT[:, q0:q0 + QS])
        im["mT"] = _bf(mt)
        in_maps.append(im)
    return in_maps


def kernel(**inputs):
    nc = _get_nc()
    ii = {k: np.asarray(v) for k, v in inputs.items()}
    in_maps = make_in_maps(ii)

    trace = bool(os.environ.get("KERNEL_TRACE"))
    res = run_bass_kernel_spmd(nc, in_maps, list(range(N_CORES)), trace=trace)
    if trace:
        kernel.last_exec_time_ns = res.exec_time_ns
        kernel.last_results = res

    out_gene = np.empty((B, G, D), np.float32)
    out_expr = np.empty((B, G, D), np.float32)
    for core in range(N_CORES):
        b, qs = core // 4, core % 4
        q0 = qs * QS
        o = res.results[core]["out"]  # (2, D, QS)
        out_gene[b, q0:q0 + QS, :] = o[0].T
        out_expr[b, q0:q0 + QS, :] = o[1].T
    return out_gene, out_expr


if __name__ == "__main__":
    t0 = time.time()
    _get_nc()
    print(f"program built in {time.time()-t0:.1f}s")


# revision 13
# speedup vs baseline: 1.2223x; 1.1504x over previous
"""Trainium2 Bass kernel for the DeepSC transformer block (B=2, G=2048, D=512, H=8).

Sharding: 8 cores = (batch b = core//4) x (query-slice qs = core%4, 512 rows).
Each core computes its 512 query rows of BOTH branches (gene + expr) end to
end; K/V projections over the full sequence are replicated inside each batch
group (no collectives). The host rotates the sequence axis per core so that
the core's own query slice is always chunk 0 -> one SPMD program.

v2 changes vs baseline:
  - host folding: fused projection eliminated (Wf premultiplied into Wq_e /
    Wk_e); key-side biases dropped (softmax-invariant); bv folded into bo.
  - scores for a head-pair land in one [128,1024] PSUM tile -> single exp.
  - reciprocal_approx_fast instead of iterative reciprocal; attention/LN
    epilogues restructured so the PE queue never waits on them.
  - weight-stationary inner loops on K projections (LDWEIGHTS reuse).
  - phase emission order P1g P2g P1e P3g P2e P3e with one global PSUM pool
    (s2 x2 = 4 banks, oacc x2 = 2, acc x2 = 2) so independent phases overlap
    and the PE stays dense (HAM stays warm).
"""
import os
import sys
import time

sys.path.insert(0, "/opt/trn_rl_repo")

import numpy as np
import ml_dtypes

import concourse.bass as bass
import concourse.tile as tile
from concourse import bacc, mybir
from concourse.bass_utils import run_bass_kernel_spmd

F32 = mybir.dt.float32
BF16 = mybir.dt.bfloat16
AF = mybir.ActivationFunctionType
ALU = mybir.AluOpType

B, G, D, H = 2, 2048, 512, 8
HD = D // H          # 64
FF = 4 * D           # 2048
N_CORES = 8
QS = G // 4          # 512 query rows per core
DT = D // 128        # 4 partition tiles over D
GT = G // 128        # 16 partition tiles over G
GC = G // 512        # 4 free-dim chunks over G
FT = FF // 128       # 16 partition tiles over FF

_NVEC_NAMES = [
    ("bq_g", DT), ("bq_e", DT),
    ("bo_g", DT), ("bo_e", DT), ("b2_g", DT), ("b2_e", DT),
    ("gamma_g1", DT), ("beta_g1", DT), ("gamma_g2", DT), ("beta_g2", DT),
    ("gamma_e1", DT), ("beta_e1", DT), ("gamma_e2", DT), ("beta_e2", DT),
    ("b1_g", FT), ("b1_e", FT),
]
_PV_COL = {}
_c = 0
for _n, _t in _NVEC_NAMES:
    _PV_COL[_n] = _c
    _c += _t
PV_NCOL = _c

_DEBUG = bool(os.environ.get("KERNEL_DEBUG"))


def _emit(nc, tc, t):
    def pv(name, i):
        c = _PV_COL[name] + i
        return pvec_sb[:, c:c + 1]

    pools = []

    def open_pool(name, space=None):
        kw = {"space": space} if space else {}
        cm = tc.tile_pool(name=name, bufs=1, **kw)
        pool = cm.__enter__()
        pools.append((name, cm))
        return pool

    def close_pool(name):
        assert pools[-1][0] == name, f"non-LIFO close {name} vs {pools[-1][0]}"
        pools.pop()[1].__exit__(None, None, None)

    pers = open_pool("persist")
    psum = open_pool("psum", space="PSUM")
    work = open_pool("work")
    attn = open_pool("attn")

    pvec_sb = pers.tile([128, PV_NCOL], F32, tag="pvec")
    nc.sync.dma_start(pvec_sb[:], t["pvec"][:])
    ones_r = pers.tile([1, 128], BF16, tag="ones_r")
    nc.vector.memset(ones_r[:], 1.0)
    ones_c = pers.tile([128, 1], BF16, tag="ones_c")
    nc.vector.memset(ones_c[:], 1.0)
    eps_sb = pers.tile([1, 1], F32, tag="eps_sb")
    nc.vector.memset(eps_sb[:], 1e-5)

    mt_sb = []
    for kt in range(GT):
        m = pers.tile([128, QS], BF16, tag="mt", bufs=GT, name=f"mt_{kt}")
        mt_sb.append(m)

    # ---------- P1 weights + embeddings (scoped; freed after P1_e) ----------
    p1pool = open_pool("p1")
    embT = {}
    w_p1 = {}

    def dma_w(pool, dname, ntile, width, tagpfx, bufs=None):
        tiles = []
        for i in range(ntile):
            w = pool.tile([128, width], BF16, tag=tagpfx,
                          bufs=bufs if bufs else ntile, name=f"{tagpfx}_{i}")
            nc.sync.dma_start(w[:], t[dname][i * 128:(i + 1) * 128, :])
            tiles.append(w)
        return tiles

    # DMA in strict need-order: gene P1 first.
    w_p1["wv_g"] = dma_w(p1pool, "wv_g", DT, D, "w_wv")
    w_p1["wk_g"] = dma_w(p1pool, "wk_g", DT, D, "w_wk")
    w_p1["wq_g"] = dma_w(p1pool, "wq_g", DT, D, "w_wq")
    embT["g"] = []
    for dt in range(DT):
        e = p1pool.tile([128, G], BF16, tag="embT", bufs=2 * DT,
                        name=f"embT_g_{dt}")
        nc.sync.dma_start(e[:], t["geneT"][dt * 128:(dt + 1) * 128, :])
        embT["g"].append(e)

    KT = {}
    QT = {}
    V = {}

    def open_attn(br):
        KT[br] = [attn.tile([128, G], BF16, tag="kt", bufs=DT,
                            name=f"KT_{br}_{i}") for i in range(DT)]
        QT[br] = [attn.tile([128, QS], BF16, tag="qt", bufs=DT,
                            name=f"QT_{br}_{i}") for i in range(DT)]
        V[br] = [attn.tile([128, H, HD + 1], BF16, tag="v", bufs=GT,
                           name=f"V_{br}_{i}") for i in range(GT)]

    def emit_p1(br):
        """V/K/Q projections for one branch."""
        wv = w_p1[f"wv_{br}"]
        if br == "g":
            ksrc = [(w_p1["wk_g"][di], embT["g"][di]) for di in range(DT)]
            qsrc = [(w_p1["wq_g"][di], embT["g"][di]) for di in range(DT)]
            bq = "bq_g"
        else:
            ksrc = [(w_p1["wke"][di],
                     embT["g"][di] if di < DT else embT["e"][di - DT])
                    for di in range(2 * DT)]
            qsrc = [(w_p1["wqe"][di],
                     embT["g"][di] if di < DT else embT["e"][di - DT])
                    for di in range(2 * DT)]
            bq = "bq_e"

        # V projection: psv[g_tile, dout] accumulated over di
        for gt in range(GT):
            psv = psum.tile([128, D], F32, tag="acc", bufs=2,
                            name=f"psv_{br}_{gt}")
            for di in range(DT):
                nc.tensor.matmul(
                    psv[:], embT[br][di][:, gt * 128:(gt + 1) * 128],
                    wv[di][:], start=(di == 0), stop=(di == DT - 1))
            nc.vector.tensor_copy(V[br][gt][:, :, 0:HD], psv[:, :])
            nc.vector.memset(V[br][gt][:, :, HD:HD + 1], 1.0)

        # K projection: weight-stationary over gc pairs
        nk = len(ksrc)
        for dt in range(DT):
            for gcp in range(2):
                ps = [psum.tile([128, 512], F32, tag="acc", bufs=2,
                                name=f"psk_{br}_{dt}_{gcp}_{x}")
                      for x in range(2)]
                for di in range(nk):
                    wti, xti = ksrc[di]
                    for g2 in range(2):
                        gc = gcp * 2 + g2
                        nc.tensor.matmul(
                            ps[g2][:], wti[:, dt * 128:(dt + 1) * 128],
                            xti[:, gc * 512:(gc + 1) * 512],
                            start=(di == 0), stop=(di == nk - 1))
                for g2 in range(2):
                    gc = gcp * 2 + g2
                    nc.vector.tensor_copy(
                        KT[br][dt][:, gc * 512:(gc + 1) * 512], ps[g2][:])

        # Q projection (own query slice only) with bias
        for dt in range(DT):
            psq = psum.tile([128, QS], F32, tag="acc", bufs=2,
                            name=f"psq_{br}_{dt}")
            for di in range(len(qsrc)):
                wti, xti = qsrc[di]
                nc.tensor.matmul(
                    psq[:], wti[:, dt * 128:(dt + 1) * 128], xti[:, 0:QS],
                    start=(di == 0), stop=(di == len(qsrc) - 1))
            nc.scalar.activation(QT[br][dt][:], psq[:], AF.Identity,
                                 bias=pv(bq, dt))

    Osc = {}

    def emit_p2(br):
        """Masked attention for one branch -> Osc[br] (normalized, bf16)."""
        Osc[br] = [work.tile([128, QS], BF16, tag="osc", bufs=2 * DT,
                             name=f"Osc_{br}_{i}") for i in range(DT)]
        for hp in range(DT):
            oacc = [psum.tile([HD + 1, QS], F32, tag="oacc", bufs=2,
                              name=f"oacc_{br}_{hp}_{x}") for x in range(2)]
            for kt in range(GT):
                s2 = psum.tile([128, 2 * QS], F32, tag="s2", bufs=2,
                               name=f"s2_{br}_{hp}_{kt}")
                for hh in range(2):
                    lo = hh * 64
                    nc.tensor.matmul(
                        s2[:, hh * QS:(hh + 1) * QS],
                        KT[br][hp][lo:lo + 64, kt * 128:(kt + 1) * 128],
                        QT[br][hp][lo:lo + 64, :],
                        start=True, stop=True, tile_position=(lo, 0))
                e2 = work.tile([128, 2 * QS], BF16, tag="e2", bufs=2,
                               name=f"e2_{br}_{hp}_{kt}")
                nc.scalar.activation(e2[:], s2[:], AF.Exp)
                em2 = work.tile([128, 2 * QS], BF16, tag="em2", bufs=2,
                                name=f"em2_{br}_{hp}_{kt}")
                for hh in range(2):
                    nc.vector.tensor_tensor(
                        em2[:, hh * QS:(hh + 1) * QS],
                        e2[:, hh * QS:(hh + 1) * QS], mt_sb[kt][:], ALU.mult)
                for hh in range(2):
                    nc.tensor.matmul(
                        oacc[hh][:], V[br][kt][:, 2 * hp + hh, :],
                        em2[:, hh * QS:(hh + 1) * QS],
                        start=(kt == 0), stop=(kt == GT - 1))
            # epilogue: normalize by 1/sum (approx reciprocal, off PE path)
            rb = psum.tile([128, QS], F32, tag="s2", bufs=2,
                           name=f"rb_{br}_{hp}")
            for hh in range(2):
                nrm = work.tile([1, QS], F32, tag="nrm", bufs=2,
                                name=f"nrm_{br}_{hp}_{hh}")
                nc.vector.tensor_copy(nrm[:], oacc[hh][HD:HD + 1, :])
                rcp = work.tile([1, QS], F32, tag="rcp", bufs=2,
                                name=f"rcp_{br}_{hp}_{hh}")
                nc.vector.reciprocal_approx_fast(out=rcp[:], in_=nrm[:])
                rcb = work.tile([1, QS], BF16, tag="rcb", bufs=2,
                                name=f"rcb_{br}_{hp}_{hh}")
                nc.vector.tensor_copy(rcb[:], rcp[:])
                nc.tensor.matmul(rb[hh * 64:hh * 64 + 64, :],
                                 ones_r[:, 0:64], rcb[:],
                                 start=True, stop=True)
            rbs = work.tile([128, QS], BF16, tag="rbs", bufs=2,
                            name=f"rbs_{br}_{hp}")
            nc.vector.tensor_copy(rbs[:], rb[:])
            for hh in range(2):
                nc.vector.tensor_tensor(
                    Osc[br][hp][hh * 64:hh * 64 + 64, :],
                    oacc[hh][0:HD, :], rbs[hh * 64:hh * 64 + 64, :], ALU.mult)

    # ---------- P3 (per branch): O proj, LN1, FFN, LN2, out ----------
    def emit_ln(x_tiles, out_writer, tagpfx):
        st_s = psum.tile([1, QS], F32, tag="acc", bufs=2, name=f"sts_{tagpfx}")
        st_q = psum.tile([1, QS], F32, tag="acc", bufs=2, name=f"stq_{tagpfx}")
        for dt in range(DT):
            xb = work.tile([128, QS], BF16, tag="xbf", bufs=2,
                           name=f"xbf_{tagpfx}_{dt}")
            nc.vector.tensor_copy(xb[:], x_tiles[dt][:])
            xq = work.tile([128, QS], BF16, tag="xsq", bufs=2,
                           name=f"xsq_{tagpfx}_{dt}")
            nc.vector.tensor_tensor(xq[:], xb[:], xb[:], ALU.mult)
            nc.tensor.matmul(st_s[:], ones_c[:], xb[:],
                             start=(dt == 0), stop=(dt == DT - 1))
            nc.tensor.matmul(st_q[:], ones_c[:], xq[:],
                             start=(dt == 0), stop=(dt == DT - 1))
        mean = work.tile([1, QS], F32, tag="lnvec", bufs=4,
                         name=f"mean_{tagpfx}")
        nc.scalar.mul(mean[:], st_s[:], 1.0 / D)
        ex2 = work.tile([1, QS], F32, tag="lnvec", bufs=4, name=f"ex2_{tagpfx}")
        nc.scalar.mul(ex2[:], st_q[:], 1.0 / D)
        msq = work.tile([1, QS], F32, tag="lnvec", bufs=4, name=f"msq_{tagpfx}")
        nc.vector.tensor_tensor(msq[:], mean[:], mean[:], ALU.mult)
        var = work.tile([1, QS], F32, tag="lnvec", bufs=4, name=f"var_{tagpfx}")
        nc.vector.tensor_tensor(var[:], ex2[:], msq[:], ALU.subtract)
        sd = work.tile([1, QS], F32, tag="lnvec", bufs=4, name=f"sd_{tagpfx}")
        nc.scalar.activation(sd[:], var[:], AF.Sqrt, bias=eps_sb[:])
        inv = work.tile([1, QS], F32, tag="lnvec", bufs=4, name=f"inv_{tagpfx}")
        nc.vector.reciprocal_approx_fast(out=inv[:], in_=sd[:])
        mi0 = work.tile([1, QS], BF16, tag="mi", bufs=2, name=f"mi0_{tagpfx}")
        nc.vector.tensor_copy(mi0[:], mean[:])
        mi1 = work.tile([1, QS], BF16, tag="mi", bufs=2, name=f"mi1_{tagpfx}")
        nc.vector.tensor_copy(mi1[:], inv[:])
        mb_ps = psum.tile([128, QS], F32, tag="acc", bufs=2,
                          name=f"mbp_{tagpfx}")
        nc.tensor.matmul(mb_ps[:], ones_r[:], mi0[:], start=True, stop=True)
        mb = work.tile([128, QS], F32, tag="bcs", bufs=2, name=f"mb_{tagpfx}")
        nc.vector.tensor_copy(mb[:], mb_ps[:])
        ib_ps = psum.tile([128, QS], F32, tag="acc", bufs=2,
                          name=f"ibp_{tagpfx}")
        nc.tensor.matmul(ib_ps[:], ones_r[:], mi1[:], start=True, stop=True)
        ib = work.tile([128, QS], F32, tag="bcs", bufs=2, name=f"ib_{tagpfx}")
        nc.vector.tensor_copy(ib[:], ib_ps[:])
        for dt in range(DT):
            t1 = work.tile([128, QS], F32, tag="lt", bufs=2,
                           name=f"lt1_{tagpfx}_{dt}")
            nc.vector.tensor_tensor(t1[:], x_tiles[dt][:], mb[:], ALU.subtract)
            t2 = work.tile([128, QS], F32, tag="lt", bufs=2,
                           name=f"lt2_{tagpfx}_{dt}")
            nc.vector.tensor_tensor(t2[:], t1[:], ib[:], ALU.mult)
            out_writer(dt, t2)

    def emit_p3a(br):
        pool = open_pool(f"p3_{br}")
        wo = dma_w(pool, f"wo_{br}", DT, D, "w_wo")
        w1 = dma_w(pool, f"w1_{br}", DT, FF, "w_w1")
        w2 = dma_w(pool, f"w2_{br}", FT, D, "w_w2")
        x1 = []
        for dt in range(DT):
            eq = work.tile([128, QS], BF16, tag="embq", bufs=2,
                           name=f"embq_{br}_{dt}")
            nc.sync.dma_start(eq[:], t[f"embq_{br}"][dt * 128:(dt + 1) * 128, :])
            psy = psum.tile([128, QS], F32, tag="acc", bufs=2,
                            name=f"psy_{br}_{dt}")
            for di in range(DT):
                nc.tensor.matmul(
                    psy[:], wo[di][:, dt * 128:(dt + 1) * 128],
                    Osc[br][di][:], start=(di == 0), stop=(di == DT - 1))
            x = work.tile([128, QS], F32, tag="x", bufs=4,
                          name=f"x1_{br}_{dt}")
            nc.vector.scalar_tensor_tensor(
                x[:], psy[:], pv(f"bo_{br}", dt), eq[:],
                ALU.add, ALU.add)
            x1.append(x)

        h_f, h_bf = [], []
        for dt in range(DT):
            h_f.append(work.tile([128, QS], F32, tag="h_f", bufs=4,
                                 name=f"h_f_{br}_{dt}"))
            h_bf.append(work.tile([128, QS], BF16, tag="h_bf", bufs=4,
                                  name=f"h_bf_{br}_{dt}"))

        def ln1_writer(dt, t2, br=br, h_f=h_f, h_bf=h_bf):
            nc.vector.tensor_scalar(
                h_f[dt][:], t2[:], pv(f"gamma_{br}1", dt),
                pv(f"beta_{br}1", dt), ALU.mult, ALU.add)
            nc.vector.tensor_copy(h_bf[dt][:], h_f[dt][:])

        emit_ln(x1, ln1_writer, f"{br}1")
        return pool, wo, w1, w2, h_f, h_bf

    def emit_p3b(br, state):
        pool, wo, w1, w2, h_f, h_bf = state
        gl = []
        for ft in range(FT):
            psu = psum.tile([128, QS], F32, tag="acc", bufs=2,
                            name=f"psu_{br}_{ft}")
            for dt in range(DT):
                nc.tensor.matmul(
                    psu[:], w1[dt][:, ft * 128:(ft + 1) * 128],
                    h_bf[dt][:], start=(dt == 0), stop=(dt == DT - 1))
            g = work.tile([128, QS], BF16, tag="gl", bufs=FT,
                          name=f"gl_{br}_{ft}")
            nc.scalar.activation(g[:], psu[:], AF.Gelu, bias=pv(f"b1_{br}", ft))
            gl.append(g)

        x2 = []
        for dt in range(DT):
            psz = psum.tile([128, QS], F32, tag="acc", bufs=2,
                            name=f"psz_{br}_{dt}")
            for ft in range(FT):
                nc.tensor.matmul(
                    psz[:], w2[ft][:, dt * 128:(dt + 1) * 128],
                    gl[ft][:], start=(ft == 0), stop=(ft == FT - 1))
            x = work.tile([128, QS], F32, tag="x", bufs=4,
                          name=f"x2_{br}_{dt}")
            nc.vector.scalar_tensor_tensor(
                x[:], psz[:], pv(f"b2_{br}", dt), h_f[dt][:],
                ALU.add, ALU.add)
            x2.append(x)

        bi = 0 if br == "g" else 1

        def ln2_writer(dt, t2, br=br, bi=bi):
            o = work.tile([128, QS], F32, tag="ot", bufs=2,
                          name=f"ot_{br}_{dt}")
            nc.vector.tensor_scalar(
                o[:], t2[:], pv(f"gamma_{br}2", dt),
                pv(f"beta_{br}2", dt), ALU.mult, ALU.add)
            nc.sync.dma_start(t["out"][bi][dt * 128:(dt + 1) * 128, :], o[:])

        emit_ln(x2, ln2_writer, f"{br}2")
        close_pool(f"p3_{br}")

    # =================== emission order ===================
    open_attn("g")
    emit_p1("g")

    # DMA for P1_e + mask while P1_g computes
    for kt in range(GT):
        nc.sync.dma_start(mt_sb[kt][:], t["mT"][kt * 128:(kt + 1) * 128, :])
    w_p1["wv_e"] = dma_w(p1pool, "wv_e", DT, D, "w_wv")
    w_p1["wke"] = dma_w(p1pool, "wke", 2 * DT, D, "w_wke")
    w_p1["wqe"] = dma_w(p1pool, "wqe", 2 * DT, D, "w_wqe")
    embT["e"] = []
    for dt in range(DT):
        e = p1pool.tile([128, G], BF16, tag="embT", bufs=2 * DT,
                        name=f"embT_e_{dt}")
        nc.sync.dma_start(e[:], t["exprT"][dt * 128:(dt + 1) * 128, :])
        embT["e"].append(e)

    emit_p2("g")
    open_attn("e")
    emit_p1("e")

    if _DEBUG:
        nc.sync.dma_start(t["dbg_kt"][:], KT["g"][0][:])
        nc.sync.dma_start(t["dbg_qt"][:], QT["g"][0][:])
        nc.sync.dma_start(t["dbg_v"][:], V["g"][0][:])
        nc.sync.dma_start(t["dbg_osc"][:], Osc["g"][0][:])

    close_pool("p1")
    st_g = emit_p3a("g")
    emit_p2("e")
    emit_p3b("g", st_g)
    st_e = emit_p3a("e")
    emit_p3b("e", st_e)
    while pools:
        pools.pop()[1].__exit__(None, None, None)


def build_program():
    nc = bacc.Bacc("TRN2", target_bir_lowering=False, debug=False,
                   num_devices=N_CORES)
    t = {}
    t["geneT"] = nc.dram_tensor("geneT", [D, G], BF16, kind="ExternalInput").ap()
    t["exprT"] = nc.dram_tensor("exprT", [D, G], BF16, kind="ExternalInput").ap()
    t["embq_g"] = nc.dram_tensor("embq_g", [D, QS], BF16, kind="ExternalInput").ap()
    t["embq_e"] = nc.dram_tensor("embq_e", [D, QS], BF16, kind="ExternalInput").ap()
    t["mT"] = nc.dram_tensor("mT", [G, QS], BF16, kind="ExternalInput").ap()
    for n in ["wq_g", "wk_g", "wv_g", "wo_g", "wv_e", "wo_e"]:
        t[n] = nc.dram_tensor(n, [D, D], BF16, kind="ExternalInput").ap()
    t["wqe"] = nc.dram_tensor("wqe", [2 * D, D], BF16, kind="ExternalInput").ap()
    t["wke"] = nc.dram_tensor("wke", [2 * D, D], BF16, kind="ExternalInput").ap()
    t["w1_g"] = nc.dram_tensor("w1_g", [D, FF], BF16, kind="ExternalInput").ap()
    t["w1_e"] = nc.dram_tensor("w1_e", [D, FF], BF16, kind="ExternalInput").ap()
    t["w2_g"] = nc.dram_tensor("w2_g", [FF, D], BF16, kind="ExternalInput").ap()
    t["w2_e"] = nc.dram_tensor("w2_e", [FF, D], BF16, kind="ExternalInput").ap()
    t["pvec"] = nc.dram_tensor("pvec", [128, PV_NCOL], F32,
                               kind="ExternalInput").ap()
    t["sel"] = nc.dram_tensor("sel", [2, 128], BF16, kind="ExternalInput").ap()
    t["out"] = nc.dram_tensor("out", [2, D, QS], F32, kind="ExternalOutput").ap()
    if _DEBUG:
        t["dbg_kt"] = nc.dram_tensor("dbg_kt", [128, G], BF16, kind="ExternalOutput").ap()
        t["dbg_qt"] = nc.dram_tensor("dbg_qt", [128, QS], BF16, kind="ExternalOutput").ap()
        t["dbg_v"] = nc.dram_tensor("dbg_v", [128, H, HD + 1], BF16, kind="ExternalOutput").ap()
        t["dbg_osc"] = nc.dram_tensor("dbg_osc", [128, QS], BF16, kind="ExternalOutput").ap()

    with tile.TileContext(nc) as tc:
        _emit(nc, tc, t)
    nc.compile()
    return nc


_NC = None


def _get_nc():
    global _NC
    if _NC is None:
        _NC = build_program()
    return _NC


def _bf(x):
    return np.ascontiguousarray(np.asarray(x, dtype=np.float32).astype(ml_dtypes.bfloat16))


def _f32(x):
    return np.ascontiguousarray(x, dtype=np.float32)


def make_in_maps(ii):
    f = {k: np.asarray(v, np.float32) for k, v in ii.items()}
    # folded weights (host, fp32 precision)
    wqe = (0.125 * f["Wq_e"]) @ f["Wf"]          # (D, 2D)
    bqe = 0.125 * (f["Wq_e"] @ f["bf"] + f["bq_e"])
    wke = f["Wk_e"] @ f["Wf"]                    # (D, 2D)
    bo_g = f["bo_g"] + f["Wo_g"] @ f["bv_gene"]
    bo_e = f["bo_e"] + f["Wo_e"] @ f["bv_expr"]

    shared = {
        "wq_g": _bf((f["Wq_g"] * 0.125).T),
        "wk_g": _bf(f["Wk_g"].T),
        "wv_g": _bf(f["Wv_gene"].T), "wo_g": _bf(f["Wo_g"].T),
        "wv_e": _bf(f["Wv_expr"].T), "wo_e": _bf(f["Wo_e"].T),
        "wqe": _bf(wqe.T), "wke": _bf(wke.T),
        "w1_g": _bf(f["W1_g"].T), "w1_e": _bf(f["W1_e"].T),
        "w2_g": _bf(f["W2_g"].T), "w2_e": _bf(f["W2_e"].T),
    }

    pvec = np.zeros((128, PV_NCOL), np.float32)

    def put(name, vec):
        c = _PV_COL[name]
        v = np.asarray(vec, np.float32)
        for i in range(v.size // 128):
            pvec[:, c + i] = v[i * 128:(i + 1) * 128]

    put("bq_g", f["bq_g"] * 0.125)
    put("bq_e", bqe)
    put("bo_g", bo_g); put("bo_e", bo_e)
    put("b2_g", f["b2_g"]); put("b2_e", f["b2_e"])
    put("gamma_g1", f["gamma_g1"]); put("beta_g1", f["beta_g1"])
    put("gamma_g2", f["gamma_g2"]); put("beta_g2", f["beta_g2"])
    put("gamma_e1", f["gamma_e1"]); put("beta_e1", f["beta_e1"])
    put("gamma_e2", f["gamma_e2"]); put("beta_e2", f["beta_e2"])
    put("b1_g", f["b1_g"]); put("b1_e", f["b1_e"])
    shared["pvec"] = pvec
    selm = np.zeros((2, 128), np.float32)
    selm[0, 0:64] = 1.0
    selm[1, 64:128] = 1.0
    shared["sel"] = _bf(selm)

    in_maps = []
    for core in range(N_CORES):
        b, qs = core // 4, core % 4
        q0 = qs * QS
        geneT = f["gene_emb"][b].T  # (D, G) fp32
        exprT = f["expr_emb"][b].T
        geneT_r = np.roll(geneT, -q0, axis=1)
        exprT_r = np.roll(exprT, -q0, axis=1)
        mt = np.roll(f["M"][b].T[:, q0:q0 + QS], -q0, axis=0)
        im = dict(shared)
        im["geneT"] = _bf(geneT_r)
        im["exprT"] = _bf(exprT_r)
        im["embq_g"] = _bf(geneT[:, q0:q0 + QS])
        im["embq_e"] = _bf(exprT[:, q0:q0 + QS])
        im["mT"] = _bf(mt)
        in_maps.append(im)
    return in_maps


def kernel(**inputs):
    nc = _get_nc()
    ii = {k: np.asarray(v) for k, v in inputs.items()}
    in_maps = make_in_maps(ii)

    trace = bool(os.environ.get("KERNEL_TRACE"))
    res = run_bass_kernel_spmd(nc, in_maps, list(range(N_CORES)), trace=trace)
    if trace:
        kernel.last_exec_time_ns = res.exec_time_ns
        kernel.last_results = res

    out_gene = np.empty((B, G, D), np.float32)
    out_expr = np.empty((B, G, D), np.float32)
    for core in range(N_CORES):
        b, qs = core // 4, core % 4
        q0 = qs * QS
        o = res.results[core]["out"]  # (2, D, QS)
        out_gene[b, q0:q0 + QS, :] = o[0].T
        out_expr[b, q0:q0 + QS, :] = o[1].T
    return out_gene, out_expr


if __name__ == "__main__":
    t0 = time.time()
    _get_nc()
    print(f"program built in {time.time()-t0:.1f}s")


# revision 19
# speedup vs baseline: 1.3411x; 1.0972x over previous
"""Trainium2 Bass kernel for the DeepSC transformer block (B=2, G=2048, D=512, H=8).

Sharding: 8 cores = (batch b = core//4) x (query-slice qs = core%4, 512 rows).
Each core computes its 512 query rows of BOTH branches (gene + expr) end to
end; K/V projections over the full sequence are replicated inside each batch
group (no collectives). The host rotates the sequence axis per core so that
the core's own query slice is always chunk 0 -> one SPMD program.

v2 changes vs baseline:
  - host folding: fused projection eliminated (Wf premultiplied into Wq_e /
    Wk_e); key-side biases dropped (softmax-invariant); bv folded into bo.
  - scores for a head-pair land in one [128,1024] PSUM tile -> single exp.
  - reciprocal_approx_fast instead of iterative reciprocal; attention/LN
    epilogues restructured so the PE queue never waits on them.
  - weight-stationary inner loops on K projections (LDWEIGHTS reuse).
  - phase emission order P1g P2g P1e P3g P2e P3e with one global PSUM pool
    (s2 x2 = 4 banks, oacc x2 = 2, acc x2 = 2) so independent phases overlap
    and the PE stays dense (HAM stays warm).
"""
import os
import sys
import time

sys.path.insert(0, "/opt/trn_rl_repo")

import numpy as np
import ml_dtypes

import concourse.bass as bass
import concourse.tile as tile
from concourse import bacc, mybir
from concourse.bass_utils import run_bass_kernel_spmd

F32 = mybir.dt.float32
BF16 = mybir.dt.bfloat16
AF = mybir.ActivationFunctionType
ALU = mybir.AluOpType

B, G, D, H = 2, 2048, 512, 8
HD = D // H          # 64
FF = 4 * D           # 2048
N_CORES = 8
QS = G // 4          # 512 query rows per core
DT = D // 128        # 4 partition tiles over D
GT = G // 128        # 16 partition tiles over G
GC = G // 512        # 4 free-dim chunks over G
FT = FF // 128       # 16 partition tiles over FF

_NVEC_NAMES = [
    ("bq_g", DT), ("bq_e", DT),
    ("bo_g", DT), ("bo_e", DT), ("b2_g", DT), ("b2_e", DT),
    ("gamma_g1", DT), ("beta_g1", DT), ("gamma_g2", DT), ("beta_g2", DT),
    ("gamma_e1", DT), ("beta_e1", DT), ("gamma_e2", DT), ("beta_e2", DT),
    ("b1_g", FT), ("b1_e", FT),
]
_PV_COL = {}
_c = 0
for _n, _t in _NVEC_NAMES:
    _PV_COL[_n] = _c
    _c += _t
PV_NCOL = _c

_DEBUG = bool(os.environ.get("KERNEL_DEBUG"))


def _emit(nc, tc, t):
    def pv(name, i):
        c = _PV_COL[name] + i
        return pvec_sb[:, c:c + 1]

    pools = []

    def open_pool(name, space=None):
        kw = {"space": space} if space else {}
        cm = tc.tile_pool(name=name, bufs=1, **kw)
        pool = cm.__enter__()
        pools.append((name, cm))
        return pool

    def close_pool(name):
        assert pools[-1][0] == name, f"non-LIFO close {name} vs {pools[-1][0]}"
        pools.pop()[1].__exit__(None, None, None)

    pers = open_pool("persist")
    psum = open_pool("psum", space="PSUM")
    work = open_pool("work")
    attn = open_pool("attn")

    pvec_sb = pers.tile([128, PV_NCOL], F32, tag="pvec")
    nc.sync.dma_start(pvec_sb[:], t["pvec"][:])
    ones_r = pers.tile([1, 128], BF16, tag="ones_r")
    nc.vector.memset(ones_r[:], 1.0)
    ones_c = pers.tile([128, 1], BF16, tag="ones_c")
    nc.vector.memset(ones_c[:], 1.0)
    eps_sb = pers.tile([1, 1], F32, tag="eps_sb")
    nc.vector.memset(eps_sb[:], 1e-5)

    mt_sb = []
    for kt in range(GT):
        m = pers.tile([128, QS], BF16, tag="mt", bufs=GT, name=f"mt_{kt}")
        mt_sb.append(m)

    # ---------- P1 weights + embeddings (scoped; freed after P1_e) ----------
    p1pool = open_pool("p1")
    embT = {}
    w_p1 = {}

    def dma_w(pool, dname, ntile, width, tagpfx, bufs=None):
        tiles = []
        for i in range(ntile):
            w = pool.tile([128, width], BF16, tag=tagpfx,
                          bufs=bufs if bufs else ntile, name=f"{tagpfx}_{i}")
            nc.sync.dma_start(w[:], t[dname][i * 128:(i + 1) * 128, :])
            tiles.append(w)
        return tiles

    # DMA in strict need-order: gene P1 first (Q weights, then K, V,
    # then embeddings column-chunked so compute starts on chunk 0).
    w_p1["wq_g"] = dma_w(p1pool, "wq_g", DT, D, "w_wq")
    embT["g"] = []
    for dt in range(DT):
        e = p1pool.tile([128, G], BF16, tag="embT", bufs=2 * DT,
                        name=f"embT_g_{dt}")
        embT["g"].append(e)
    for dt in range(DT):
        nc.sync.dma_start(embT["g"][dt][:, 0:512],
                          t["geneT"][dt * 128:(dt + 1) * 128, 0:512])
    w_p1["wk_g"] = dma_w(p1pool, "wk_g", DT, D, "w_wk")
    w_p1["wv_g"] = dma_w(p1pool, "wv_g", DT, D, "w_wv")
    for c in range(1, GC):
        for dt in range(DT):
            nc.sync.dma_start(
                embT["g"][dt][:, c * 512:(c + 1) * 512],
                t["geneT"][dt * 128:(dt + 1) * 128, c * 512:(c + 1) * 512])

    KT = {}
    QT = {}
    V = {}

    def open_attn(br):
        KT[br] = [attn.tile([128, G], BF16, tag="kt", bufs=DT,
                            name=f"KT_{br}_{i}") for i in range(DT)]
        QT[br] = [attn.tile([128, QS], BF16, tag="qt", bufs=DT,
                            name=f"QT_{br}_{i}") for i in range(DT)]
        V[br] = [attn.tile([128, H, HD + 1], BF16, tag="v", bufs=GT,
                           name=f"V_{br}_{i}") for i in range(GT)]

    def emit_p1(br):
        """V/K/Q projections for one branch."""
        wv = w_p1[f"wv_{br}"]
        if br == "g":
            ksrc = [(w_p1["wk_g"][di], embT["g"][di]) for di in range(DT)]
            qsrc = [(w_p1["wq_g"][di], embT["g"][di]) for di in range(DT)]
            bq = "bq_g"
        else:
            ksrc = [(w_p1["wke"][di],
                     embT["g"][di] if di < DT else embT["e"][di - DT])
                    for di in range(2 * DT)]
            qsrc = [(w_p1["wqe"][di],
                     embT["g"][di] if di < DT else embT["e"][di - DT])
                    for di in range(2 * DT)]
            bq = "bq_e"

        # Q projection (own query slice only) with bias
        for dt in range(DT):
            psq = psum.tile([128, QS], F32, tag="acc", bufs=2,
                            name=f"psq_{br}_{dt}")
            for di in range(len(qsrc)):
                wti, xti = qsrc[di]
                nc.tensor.matmul(
                    psq[:], wti[:, dt * 128:(dt + 1) * 128], xti[:, 0:QS],
                    start=(di == 0), stop=(di == len(qsrc) - 1))
            nc.scalar.activation(QT[br][dt][:], psq[:], AF.Identity,
                                 bias=pv(bq, dt))

        # K projection: weight-stationary over gc pairs
        nk = len(ksrc)
        for dt in range(DT):
            for gcp in range(2):
                ps = [psum.tile([128, 512], F32, tag="acc", bufs=2,
                                name=f"psk_{br}_{dt}_{gcp}_{x}")
                      for x in range(2)]
                for di in range(nk):
                    wti, xti = ksrc[di]
                    for g2 in range(2):
                        gc = gcp * 2 + g2
                        nc.tensor.matmul(
                            ps[g2][:], wti[:, dt * 128:(dt + 1) * 128],
                            xti[:, gc * 512:(gc + 1) * 512],
                            start=(di == 0), stop=(di == nk - 1))
                for g2 in range(2):
                    gc = gcp * 2 + g2
                    if br == "g":
                        nc.scalar.copy(
                            KT[br][dt][:, gc * 512:(gc + 1) * 512], ps[g2][:])
                    else:
                        nc.vector.tensor_copy(
                            KT[br][dt][:, gc * 512:(gc + 1) * 512], ps[g2][:])

        # V projection: psv[g_tile, dout] accumulated over di
        for gt in range(GT):
            psv = psum.tile([128, D], F32, tag="acc", bufs=2,
                            name=f"psv_{br}_{gt}")
            for di in range(DT):
                nc.tensor.matmul(
                    psv[:], embT[br][di][:, gt * 128:(gt + 1) * 128],
                    wv[di][:], start=(di == 0), stop=(di == DT - 1))
            if br == "g":
                nc.scalar.copy(V[br][gt][:, :, 0:HD], psv[:, :])
            else:
                nc.vector.tensor_copy(V[br][gt][:, :, 0:HD], psv[:, :])
            nc.vector.memset(V[br][gt][:, :, HD:HD + 1], 1.0)

    Osc = {}

    def emit_p2(br):
        """Masked attention for one branch -> Osc[br] (normalized, bf16)."""
        Osc[br] = [work.tile([128, QS], BF16, tag="osc", bufs=2 * DT,
                             name=f"Osc_{br}_{i}") for i in range(DT)]
        for hp in range(DT):
            oacc = [psum.tile([HD + 1, QS], F32, tag="oacc", bufs=2,
                              name=f"oacc_{br}_{hp}_{x}") for x in range(2)]
            for kt in range(GT):
                s2 = psum.tile([128, 2 * QS], F32, tag="s2", bufs=2,
                               name=f"s2_{br}_{hp}_{kt}")
                for hh in range(2):
                    lo = hh * 64
                    nc.tensor.matmul(
                        s2[:, hh * QS:(hh + 1) * QS],
                        KT[br][hp][lo:lo + 64, kt * 128:(kt + 1) * 128],
                        QT[br][hp][lo:lo + 64, :],
                        start=True, stop=True, tile_position=(lo, 0))
                e2 = work.tile([128, 2 * QS], BF16, tag="e2", bufs=2,
                               name=f"e2_{br}_{hp}_{kt}")
                nc.scalar.activation(e2[:], s2[:], AF.Exp)
                em2 = work.tile([128, 2 * QS], BF16, tag="em2", bufs=2,
                                name=f"em2_{br}_{hp}_{kt}")
                for hh in range(2):
                    nc.vector.tensor_tensor(
                        em2[:, hh * QS:(hh + 1) * QS],
                        e2[:, hh * QS:(hh + 1) * QS], mt_sb[kt][:], ALU.mult)
                for hh in range(2):
                    nc.tensor.matmul(
                        oacc[hh][:], V[br][kt][:, 2 * hp + hh, :],
                        em2[:, hh * QS:(hh + 1) * QS],
                        start=(kt == 0), stop=(kt == GT - 1))
            # epilogue: normalize by 1/sum (approx reciprocal, off PE path)
            rb = psum.tile([128, QS], F32, tag="s2", bufs=2,
                           name=f"rb_{br}_{hp}")
            for hh in range(2):
                nrm = work.tile([1, QS], F32, tag="nrm", bufs=2,
                                name=f"nrm_{br}_{hp}_{hh}")
                nc.vector.tensor_copy(nrm[:], oacc[hh][HD:HD + 1, :])
                rcp = work.tile([1, QS], F32, tag="rcp", bufs=2,
                                name=f"rcp_{br}_{hp}_{hh}")
                nc.vector.reciprocal_approx_fast(out=rcp[:], in_=nrm[:])
                rcb = work.tile([1, QS], BF16, tag="rcb", bufs=2,
                                name=f"rcb_{br}_{hp}_{hh}")
                nc.vector.tensor_copy(rcb[:], rcp[:])
                nc.tensor.matmul(rb[hh * 64:hh * 64 + 64, :],
                                 ones_r[:, 0:64], rcb[:],
                                 start=True, stop=True)
            rbs = work.tile([128, QS], BF16, tag="rbs", bufs=2,
                            name=f"rbs_{br}_{hp}")
            nc.vector.tensor_copy(rbs[:], rb[:])
            for hh in range(2):
                nc.vector.tensor_tensor(
                    Osc[br][hp][hh * 64:hh * 64 + 64, :],
                    oacc[hh][0:HD, :], rbs[hh * 64:hh * 64 + 64, :], ALU.mult)

    # ---------- P3 (per branch): O proj, LN1, FFN, LN2, out ----------
    def emit_ln(x_tiles, out_writer, tagpfx):
        st_s = psum.tile([1, QS], F32, tag="acc", bufs=2, name=f"sts_{tagpfx}")
        st_q = psum.tile([1, QS], F32, tag="acc", bufs=2, name=f"stq_{tagpfx}")
        for dt in range(DT):
            xb = x_tiles[dt]
            xq = work.tile([128, QS], BF16, tag="xsq", bufs=2,
                           name=f"xsq_{tagpfx}_{dt}")
            nc.vector.tensor_tensor(xq[:], xb[:], xb[:], ALU.mult)
            nc.tensor.matmul(st_s[:], ones_c[:], xb[:],
                             start=(dt == 0), stop=(dt == DT - 1))
            nc.tensor.matmul(st_q[:], ones_c[:], xq[:],
                             start=(dt == 0), stop=(dt == DT - 1))
        mean = work.tile([1, QS], F32, tag="lnvec", bufs=4,
                         name=f"mean_{tagpfx}")
        nc.scalar.mul(mean[:], st_s[:], 1.0 / D)
        ex2 = work.tile([1, QS], F32, tag="lnvec", bufs=4, name=f"ex2_{tagpfx}")
        nc.scalar.mul(ex2[:], st_q[:], 1.0 / D)
        msq = work.tile([1, QS], F32, tag="lnvec", bufs=4, name=f"msq_{tagpfx}")
        nc.vector.tensor_tensor(msq[:], mean[:], mean[:], ALU.mult)
        var = work.tile([1, QS], F32, tag="lnvec", bufs=4, name=f"var_{tagpfx}")
        nc.vector.tensor_tensor(var[:], ex2[:], msq[:], ALU.subtract)
        sd = work.tile([1, QS], F32, tag="lnvec", bufs=4, name=f"sd_{tagpfx}")
        nc.scalar.activation(sd[:], var[:], AF.Sqrt, bias=eps_sb[:])
        inv = work.tile([1, QS], F32, tag="lnvec", bufs=4, name=f"inv_{tagpfx}")
        nc.vector.reciprocal_approx_fast(out=inv[:], in_=sd[:])
        mi0 = work.tile([1, QS], BF16, tag="mi", bufs=2, name=f"mi0_{tagpfx}")
        nc.vector.tensor_copy(mi0[:], mean[:])
        mi1 = work.tile([1, QS], BF16, tag="mi", bufs=2, name=f"mi1_{tagpfx}")
        nc.vector.tensor_copy(mi1[:], inv[:])
        mb_ps = psum.tile([128, QS], F32, tag="acc", bufs=2,
                          name=f"mbp_{tagpfx}")
        nc.tensor.matmul(mb_ps[:], ones_r[:], mi0[:], start=True, stop=True)
        mb = work.tile([128, QS], F32, tag="bcs", bufs=2, name=f"mb_{tagpfx}")
        nc.vector.tensor_copy(mb[:], mb_ps[:])
        ib_ps = psum.tile([128, QS], F32, tag="acc", bufs=2,
                          name=f"ibp_{tagpfx}")
        nc.tensor.matmul(ib_ps[:], ones_r[:], mi1[:], start=True, stop=True)
        ib = work.tile([128, QS], F32, tag="bcs", bufs=2, name=f"ib_{tagpfx}")
        nc.vector.tensor_copy(ib[:], ib_ps[:])
        for dt in range(DT):
            t1 = work.tile([128, QS], F32, tag="lt", bufs=2,
                           name=f"lt1_{tagpfx}_{dt}")
            nc.vector.tensor_tensor(t1[:], x_tiles[dt][:], mb[:], ALU.subtract)
            t2 = work.tile([128, QS], F32, tag="lt", bufs=2,
                           name=f"lt2_{tagpfx}_{dt}")
            nc.vector.tensor_tensor(t2[:], t1[:], ib[:], ALU.mult)
            out_writer(dt, t2)

    def emit_p3a(br):
        pool = open_pool(f"p3_{br}")
        wo = dma_w(pool, f"wo_{br}", DT, D, "w_wo")
        w1 = dma_w(pool, f"w1_{br}", DT, FF, "w_w1")
        w2 = dma_w(pool, f"w2_{br}", FT, D, "w_w2")
        x1 = []
        for dt in range(DT):
            eq = work.tile([128, QS], BF16, tag="embq", bufs=2,
                           name=f"embq_{br}_{dt}")
            nc.sync.dma_start(eq[:], t[f"embq_{br}"][dt * 128:(dt + 1) * 128, :])
            psy = psum.tile([128, QS], F32, tag="acc", bufs=2,
                            name=f"psy_{br}_{dt}")
            for di in range(DT):
                nc.tensor.matmul(
                    psy[:], wo[di][:, dt * 128:(dt + 1) * 128],
                    Osc[br][di][:], start=(di == 0), stop=(di == DT - 1))
            x = work.tile([128, QS], BF16, tag="x", bufs=4,
                          name=f"x1_{br}_{dt}")
            nc.vector.scalar_tensor_tensor(
                x[:], psy[:], pv(f"bo_{br}", dt), eq[:],
                ALU.add, ALU.add)
            x1.append(x)

        h_bf = []
        for dt in range(DT):
            h_bf.append(work.tile([128, QS], BF16, tag="h_bf", bufs=4,
                                  name=f"h_bf_{br}_{dt}"))

        def ln1_writer(dt, t2, br=br, h_bf=h_bf):
            nc.vector.tensor_scalar(
                h_bf[dt][:], t2[:], pv(f"gamma_{br}1", dt),
                pv(f"beta_{br}1", dt), ALU.mult, ALU.add)

        emit_ln(x1, ln1_writer, f"{br}1")
        h_f = h_bf
        return pool, wo, w1, w2, h_f, h_bf

    def emit_p3b(br, state):
        pool, wo, w1, w2, h_f, h_bf = state
        gl = []
        for ft in range(FT):
            psu = psum.tile([128, QS], F32, tag="acc", bufs=2,
                            name=f"psu_{br}_{ft}")
            for dt in range(DT):
                nc.tensor.matmul(
                    psu[:], w1[dt][:, ft * 128:(ft + 1) * 128],
                    h_bf[dt][:], start=(dt == 0), stop=(dt == DT - 1))
            g = work.tile([128, QS], BF16, tag="gl", bufs=FT,
                          name=f"gl_{br}_{ft}")
            nc.scalar.activation(g[:], psu[:], AF.Gelu, bias=pv(f"b1_{br}", ft))
            gl.append(g)

        x2 = []
        for dt in range(DT):
            psz = psum.tile([128, QS], F32, tag="acc", bufs=2,
                            name=f"psz_{br}_{dt}")
            for ft in range(FT):
                nc.tensor.matmul(
                    psz[:], w2[ft][:, dt * 128:(dt + 1) * 128],
                    gl[ft][:], start=(ft == 0), stop=(ft == FT - 1))
            x = work.tile([128, QS], BF16, tag="x", bufs=4,
                          name=f"x2_{br}_{dt}")
            nc.vector.scalar_tensor_tensor(
                x[:], psz[:], pv(f"b2_{br}", dt), h_f[dt][:],
                ALU.add, ALU.add)
            x2.append(x)

        bi = 0 if br == "g" else 1

        def ln2_writer(dt, t2, br=br, bi=bi):
            o = work.tile([128, QS], F32, tag="ot", bufs=2,
                          name=f"ot_{br}_{dt}")
            nc.vector.tensor_scalar(
                o[:], t2[:], pv(f"gamma_{br}2", dt),
                pv(f"beta_{br}2", dt), ALU.mult, ALU.add)
            nc.sync.dma_start(t["out"][bi][dt * 128:(dt + 1) * 128, :], o[:])

        emit_ln(x2, ln2_writer, f"{br}2")
        close_pool(f"p3_{br}")

    def emit_p3b_fused(br, state):
        pool, wo, w1, w2, h_f, h_bf = state
        psz = []
        for pz in range(2):
            z = psum.tile([128, 2 * QS], F32, tag="s2", bufs=2,
                          name=f"psz2_{br}_{pz}")
            psz.append(z)
        for ft in range(FT):
            psu = psum.tile([128, QS], F32, tag="acc", bufs=2,
                            name=f"psu_{br}_{ft}")
            for dt in range(DT):
                nc.tensor.matmul(
                    psu[:], w1[dt][:, ft * 128:(ft + 1) * 128],
                    h_bf[dt][:], start=(dt == 0), stop=(dt == DT - 1))
            g = work.tile([128, QS], BF16, tag="gl", bufs=FT,
                          name=f"gl_{br}_{ft}")
            nc.scalar.activation(g[:], psu[:], AF.Gelu, bias=pv(f"b1_{br}", ft))
            for dt in range(DT):
                nc.tensor.matmul(
                    psz[dt // 2][:, (dt % 2) * QS:(dt % 2 + 1) * QS],
                    w2[ft][:, dt * 128:(dt + 1) * 128], g[:],
                    start=(ft == 0), stop=(ft == FT - 1))
        x2 = []
        for dt in range(DT):
            x = work.tile([128, QS], BF16, tag="x", bufs=4,
                          name=f"x2_{br}_{dt}")
            nc.vector.scalar_tensor_tensor(
                x[:], psz[dt // 2][:, (dt % 2) * QS:(dt % 2 + 1) * QS],
                pv(f"b2_{br}", dt), h_f[dt][:], ALU.add, ALU.add)
            x2.append(x)

        bi = 0 if br == "g" else 1

        def ln2_writer(dt, t2, br=br, bi=bi):
            o = work.tile([128, QS], F32, tag="ot", bufs=2,
                          name=f"ot_{br}_{dt}")
            nc.vector.tensor_scalar(
                o[:], t2[:], pv(f"gamma_{br}2", dt),
                pv(f"beta_{br}2", dt), ALU.mult, ALU.add)
            nc.sync.dma_start(t["out"][bi][dt * 128:(dt + 1) * 128, :], o[:])

        emit_ln(x2, ln2_writer, f"{br}2")
        close_pool(f"p3_{br}")

    # =================== emission order ===================
    open_attn("g")
    emit_p1("g")

    # DMA for P1_e + mask while P1_g computes
    for kt in range(GT):
        nc.sync.dma_start(mt_sb[kt][:], t["mT"][kt * 128:(kt + 1) * 128, :])
    w_p1["wqe"] = dma_w(p1pool, "wqe", 2 * DT, D, "w_wqe")
    w_p1["wv_e"] = dma_w(p1pool, "wv_e", DT, D, "w_wv")
    w_p1["wke"] = dma_w(p1pool, "wke", 2 * DT, D, "w_wke")
    embT["e"] = []
    for dt in range(DT):
        e = p1pool.tile([128, G], BF16, tag="embT", bufs=2 * DT,
                        name=f"embT_e_{dt}")
        nc.sync.dma_start(e[:], t["exprT"][dt * 128:(dt + 1) * 128, :])
        embT["e"].append(e)

    emit_p2("g")
    open_attn("e")
    emit_p1("e")

    if _DEBUG:
        nc.sync.dma_start(t["dbg_kt"][:], KT["g"][0][:])
        nc.sync.dma_start(t["dbg_qt"][:], QT["g"][0][:])
        nc.sync.dma_start(t["dbg_v"][:], V["g"][0][:])
        nc.sync.dma_start(t["dbg_osc"][:], Osc["g"][0][:])

    close_pool("p1")
    st_g = emit_p3a("g")
    emit_p2("e")
    emit_p3b("g", st_g)
    st_e = emit_p3a("e")
    emit_p3b("e", st_e)
    while pools:
        pools.pop()[1].__exit__(None, None, None)


def build_program():
    nc = bacc.Bacc("TRN2", target_bir_lowering=False, debug=False,
                   num_devices=N_CORES)
    t = {}
    t["geneT"] = nc.dram_tensor("geneT", [D, G], BF16, kind="ExternalInput").ap()
    t["exprT"] = nc.dram_tensor("exprT", [D, G], BF16, kind="ExternalInput").ap()
    t["embq_g"] = nc.dram_tensor("embq_g", [D, QS], BF16, kind="ExternalInput").ap()
    t["embq_e"] = nc.dram_tensor("embq_e", [D, QS], BF16, kind="ExternalInput").ap()
    t["mT"] = nc.dram_tensor("mT", [G, QS], BF16, kind="ExternalInput").ap()
    for n in ["wq_g", "wk_g", "wv_g", "wo_g", "wv_e", "wo_e"]:
        t[n] = nc.dram_tensor(n, [D, D], BF16, kind="ExternalInput").ap()
    t["wqe"] = nc.dram_tensor("wqe", [2 * D, D], BF16, kind="ExternalInput").ap()
    t["wke"] = nc.dram_tensor("wke", [2 * D, D], BF16, kind="ExternalInput").ap()
    t["w1_g"] = nc.dram_tensor("w1_g", [D, FF], BF16, kind="ExternalInput").ap()
    t["w1_e"] = nc.dram_tensor("w1_e", [D, FF], BF16, kind="ExternalInput").ap()
    t["w2_g"] = nc.dram_tensor("w2_g", [FF, D], BF16, kind="ExternalInput").ap()
    t["w2_e"] = nc.dram_tensor("w2_e", [FF, D], BF16, kind="ExternalInput").ap()
    t["pvec"] = nc.dram_tensor("pvec", [128, PV_NCOL], F32,
                               kind="ExternalInput").ap()
    t["sel"] = nc.dram_tensor("sel", [2, 128], BF16, kind="ExternalInput").ap()
    t["out"] = nc.dram_tensor("out", [2, D, QS], F32, kind="ExternalOutput").ap()
    if _DEBUG:
        t["dbg_kt"] = nc.dram_tensor("dbg_kt", [128, G], BF16, kind="ExternalOutput").ap()
        t["dbg_qt"] = nc.dram_tensor("dbg_qt", [128, QS], BF16, kind="ExternalOutput").ap()
        t["dbg_v"] = nc.dram_tensor("dbg_v", [128, H, HD + 1], BF16, kind="ExternalOutput").ap()
        t["dbg_osc"] = nc.dram_tensor("dbg_osc", [128, QS], BF16, kind="ExternalOutput").ap()

    with tile.TileContext(nc) as tc:
        _emit(nc, tc, t)
    nc.compile()
    return nc


_NC = None


def _get_nc():
    global _NC
    if _NC is None:
        _NC = build_program()
    return _NC


def _bf(x):
    return np.ascontiguousarray(np.asarray(x, dtype=np.float32).astype(ml_dtypes.bfloat16))


def _f32(x):
    return np.ascontiguousarray(x, dtype=np.float32)


def make_in_maps(ii):
    f = {k: np.asarray(v, np.float32) for k, v in ii.items()}
    # folded weights (host, fp32 precision)
    wqe = (0.125 * f["Wq_e"]) @ f["Wf"]          # (D, 2D)
    bqe = 0.125 * (f["Wq_e"] @ f["bf"] + f["bq_e"])
    wke = f["Wk_e"] @ f["Wf"]                    # (D, 2D)
    bo_g = f["bo_g"] + f["Wo_g"] @ f["bv_gene"]
    bo_e = f["bo_e"] + f["Wo_e"] @ f["bv_expr"]

    shared = {
        "wq_g": _bf((f["Wq_g"] * 0.125).T),
        "wk_g": _bf(f["Wk_g"].T),
        "wv_g": _bf(f["Wv_gene"].T), "wo_g": _bf(f["Wo_g"].T),
        "wv_e": _bf(f["Wv_expr"].T), "wo_e": _bf(f["Wo_e"].T),
        "wqe": _bf(wqe.T), "wke": _bf(wke.T),
        "w1_g": _bf(f["W1_g"].T), "w1_e": _bf(f["W1_e"].T),
        "w2_g": _bf(f["W2_g"].T), "w2_e": _bf(f["W2_e"].T),
    }

    pvec = np.zeros((128, PV_NCOL), np.float32)

    def put(name, vec):
        c = _PV_COL[name]
        v = np.asarray(vec, np.float32)
        for i in range(v.size // 128):
            pvec[:, c + i] = v[i * 128:(i + 1) * 128]

    put("bq_g", f["bq_g"] * 0.125)
    put("bq_e", bqe)
    put("bo_g", bo_g); put("bo_e", bo_e)
    put("b2_g", f["b2_g"]); put("b2_e", f["b2_e"])
    put("gamma_g1", f["gamma_g1"]); put("beta_g1", f["beta_g1"])
    put("gamma_g2", f["gamma_g2"]); put("beta_g2", f["beta_g2"])
    put("gamma_e1", f["gamma_e1"]); put("beta_e1", f["beta_e1"])
    put("gamma_e2", f["gamma_e2"]); put("beta_e2", f["beta_e2"])
    put("b1_g", f["b1_g"]); put("b1_e", f["b1_e"])
    shared["pvec"] = pvec
    selm = np.zeros((2, 128), np.float32)
    selm[0, 0:64] = 1.0
    selm[1, 64:128] = 1.0
    shared["sel"] = _bf(selm)

    in_maps = []
    for core in range(N_CORES):
        b, qs = core // 4, core % 4
        q0 = qs * QS
        geneT = f["gene_emb"][b].T  # (D, G) fp32
        exprT = f["expr_emb"][b].T
        geneT_r = np.roll(geneT, -q0, axis=1)
        exprT_r = np.roll(exprT, -q0, axis=1)
        mt = np.roll(f["M"][b].T[:, q0:q0 + QS], -q0, axis=0)
        im = dict(shared)
        im["geneT"] = _bf(geneT_r)
        im["exprT"] = _bf(exprT_r)
        im["embq_g"] = _bf(geneT[:, q0:q0 + QS])
        im["embq_e"] = _bf(exprT[:, q0:q0 + QS])
        im["mT"] = _bf(mt)
        in_maps.append(im)
    return in_maps


def kernel(**inputs):
    nc = _get_nc()
    ii = {k: np.asarray(v) for k, v in inputs.items()}
    in_maps = make_in_maps(ii)

    trace = bool(os.environ.get("KERNEL_TRACE"))
    res = run_bass_kernel_spmd(nc, in_maps, list(range(N_CORES)), trace=trace)
    if trace:
        kernel.last_exec_time_ns = res.exec_time_ns
        kernel.last_results = res

    out_gene = np.empty((B, G, D), np.float32)
    out_expr = np.empty((B, G, D), np.float32)
    for core in range(N_CORES):
        b, qs = core // 4, core % 4
        q0 = qs * QS
        o = res.results[core]["out"]  # (2, D, QS)
        out_gene[b, q0:q0 + QS, :] = o[0].T
        out_expr[b, q0:q0 + QS, :] = o[1].T
    return out_gene, out_expr


if __name__ == "__main__":
    t0 = time.time()
    _get_nc()
    print(f"program built in {time.time()-t0:.1f}s")
